# revision 33
# baseline (speedup 1.0000x reference)
"""Trainium2 Bass kernel for the MultiHeadAttention transformer block.

Sharding: 8 cores, core c handles batch b=c//2 and query-row half
(c%2)*1024 .. +1024, all 8 heads.  Each core is fully independent
(no collectives).

Layout strategy: everything lives transposed on chip — [channel/d on
partitions, sequence on free dim] — from the input loads through the
final LayerNorm, so no on-chip transposes are ever needed:
  - scores are computed as S^T[lk, lq]; the key mask is a
    per-partition bias folded into the Exp activation,
  - K^T is stored zero-padded to the full 128 contraction rows (even
    heads in rows 0:64, odd heads in rows 64:128, zeros elsewhere) so
    every QK matmul runs with k=128 and full PE-array activity (the
    HAM clock gate watches array utilization),
  - A@V runs with V stationary producing attn^T directly; softmax
    row-sums come from an appended ones-column on V and are applied
    via a DRAM-bounced partition-broadcast of the row reciprocals,
  - LayerNorm mean/var come from ones-column matmuls over the channel
    (partition) dim on bf16 copies; gamma/beta/fc-bias are
    per-partition scalars in this layout; the whole LN0 -> fc -> LN1
    tail is pipelined in two independent 512-query halves,
  - the fc output projection consumes LN0^T directly and produces
    out^T, which the host un-transposes for free.
"""

import sys

if "/opt/trn_rl_repo" not in sys.path:
    sys.path.insert(0, "/opt/trn_rl_repo")

import numpy as np

import concourse.bacc as bacc
import concourse.bass as bass
import concourse.tile as tile
from concourse import mybir
from concourse.bass_utils import run_bass_kernel_spmd

H, D, DK, DV = 8, 512, 64, 64
B, L = 4, 2048
P = 128
LQ = L // 2          # query rows per core
NCORES = 8
EPS = 1e-5
NEG = -1e9 / 8.0     # masked score after the /temperature divide
F32 = mybir.dt.float32
BF16 = mybir.dt.bfloat16
AF = mybir.ActivationFunctionType
Alu = mybir.AluOpType

DT = D // P     # 4 d-tiles
LKT = L // P    # 16 key tiles
NB = LQ // 512  # 2 psum-bank columns of queries

_CACHE = {}


def _bcast(ap, parts):
    """Partition-broadcast view of a [1, n] DRAM AP for DMA replication."""
    return ap.to_broadcast([parts] + list(ap.shape[1:]))


def _emit(nc, tc):
    qT = nc.dram_tensor("qT", [P, DT, LQ], BF16, kind="ExternalInput")
    kT = nc.dram_tensor("kT", [P, DT, L], BF16, kind="ExternalInput")
    vT = nc.dram_tensor("vT", [P, DT, L], BF16, kind="ExternalInput")
    qresT = nc.dram_tensor("qresT", [P, DT, LQ], F32, kind="ExternalInput")
    WqT = nc.dram_tensor("WqT", [P, DT, D], BF16, kind="ExternalInput")
    WkT = nc.dram_tensor("WkT", [P, DT, D], BF16, kind="ExternalInput")
    WvT = nc.dram_tensor("WvT", [P, DT, D], BF16, kind="ExternalInput")
    fcwT = nc.dram_tensor("fcwT", [P, DT, D], BF16, kind="ExternalInput")
    mb = nc.dram_tensor("mb", [P, H * LKT], F32, kind="ExternalInput")
    vecs = nc.dram_tensor("vecs", [5, P, DT], F32, kind="ExternalInput")
    out = nc.dram_tensor("out", [P, DT, LQ], F32, kind="ExternalOutput")

    with (
        tc.tile_pool(name="consts", bufs=1) as consts,
        tc.tile_pool(name="projout", bufs=1) as projout,
        tc.tile_pool(name="dramp", bufs=3, space="DRAM") as dramp,
        tc.tile_pool(name="psA", bufs=2, space="PSUM") as psProj,
    ):
        # ---- constants resident for the whole kernel ----
        mb_s = consts.tile([P, H * LKT], F32)
        nc.sync.dma_start(out=mb_s, in_=mb[:, :])
        gbT = consts.tile([P, 5, DT], F32)   # g0,b0,g1,b1,fcb as [p, dt]
        for i in range(5):
            nc.sync.dma_start(out=gbT[:, i, :], in_=vecs[i, :, :])
        eps_t = consts.tile([P, 1], F32)
        nc.vector.memset(eps_t, EPS)
        ones_st = consts.tile([P, 1], BF16)  # stats reduction column
        nc.vector.memset(ones_st, 1.0)
        ones_r1 = consts.tile([1, P], BF16)  # rank-1 broadcast row
        nc.vector.memset(ones_r1, 1.0)
        warm = consts.tile([P, 512], BF16)   # PE warm-up fodder
        nc.vector.memset(warm[:, :], 0.0)

        # ---- persistent big tiles ----
        QT_s = projout.tile([P, DT, LQ], BF16)         # Q^T  [ch, lq]
        KTz_s = projout.tile([P, H, L], BF16)          # zero-padded K^T
        V_s = projout.tile([P, LKT, H, DV + 1], BF16)  # V rows + ones col
        attnT = projout.tile([P, DT, LQ], F32)         # attention out ^T
        qresT_s = projout.tile([P, DT, LQ], F32)

        with tc.tile_pool(name="statin", bufs=1) as statin:
            xbf = statin.tile([P, DT, LQ], BF16, tag="xbf")
            x2bf = statin.tile([P, DT, LQ], BF16, tag="x2bf")
            s1a = [statin.tile([1, 512], F32, tag=f"s1a{j}", name=f"s1a{j}")
                   for j in range(NB)]
            s2a = [statin.tile([1, 512], F32, tag=f"s2a{j}", name=f"s2a{j}")
                   for j in range(NB)]
            for j in range(NB):
                nc.vector.memset(s1a[j], 0.0)
                nc.vector.memset(s2a[j], 0.0)

            # ====== phases A+B fused: per-pair projections + attention ======
            with (
                tc.tile_pool(name="inp", bufs=1) as inp,
                tc.tile_pool(name="wts", bufs=1) as wts,
                tc.tile_pool(name="pT", bufs=8) as pTp,
                tc.tile_pool(name="rcp", bufs=3) as rcp,
                tc.tile_pool(name="bcsp", bufs=3) as bcsp,
            ):
                for w in range(24):
                    wps = psProj.tile([P, 512], F32, tag="proj",
                                      name=f"warm{w}")
                    nc.tensor.matmul(wps[:, :], warm[:, 0:P], warm[:, :],
                                     start=True, stop=True)
                WqT_s = wts.tile([P, DT, D], BF16)
                qT_s = inp.tile([P, DT, LQ], BF16)
                WkT_s = wts.tile([P, DT, D], BF16)
                kT_s = inp.tile([P, DT, L], BF16)
                WvT_s = wts.tile([P, DT, D], BF16)
                vT_s = inp.tile([P, DT, L], BF16)
                for dt in range(DT):
                    nc.sync.dma_start(out=WkT_s[:, dt, :], in_=WkT[:, dt, :])
                    nc.sync.dma_start(out=kT_s[:, dt, :], in_=kT[:, dt, :])
                for dt in range(DT):
                    nc.sync.dma_start(out=WqT_s[:, dt, :], in_=WqT[:, dt, :])
                    nc.sync.dma_start(out=qT_s[:, dt, :], in_=qT[:, dt, :])
                for dt in range(DT):
                    nc.sync.dma_start(out=WvT_s[:, dt, :], in_=WvT[:, dt, :])
                    nc.sync.dma_start(out=vT_s[:, dt, :], in_=vT[:, dt, :])
                for dt in range(DT):
                    nc.sync.dma_start(out=qresT_s[:, dt, :],
                                      in_=qresT[:, dt, :])

                # zero the pad rows of KTz: even heads live in rows 0:64,
                # odd heads in rows 64:128 — zero the other half.
                nc.vector.memset(KTz_s[64:128, 0:H:2, :], 0.0)
                nc.vector.memset(KTz_s[0:64, 1:H:2, :], 0.0)
                nc.vector.memset(V_s[:, :, :, DV:DV + 1], 1.0)

                def emit_qproj(m):
                    pss = [psProj.tile([P, 512], F32, tag="proj",
                                       name=f"psq{m}_{j}") for j in range(NB)]
                    for dt in range(DT):
                        for jb in range(NB):
                            nc.tensor.matmul(
                                pss[jb][:, :],
                                WqT_s[:, dt, m * P:(m + 1) * P],
                                qT_s[:, dt, jb * 512:(jb + 1) * 512],
                                start=(dt == 0), stop=(dt == DT - 1))
                    for jb in range(NB):
                        nc.vector.tensor_copy(
                            QT_s[:, m, jb * 512:(jb + 1) * 512], pss[jb][:, :])

                def emit_kproj(m):
                    for jp in range(2):
                        pss = [psProj.tile([P, 512], F32, tag="proj",
                                           name=f"psk{m}_{jp}_{j}")
                               for j in range(2)]
                        for dt in range(DT):
                            for ji in range(2):
                                jb = 2 * jp + ji
                                nc.tensor.matmul(
                                    pss[ji][:, :],
                                    WkT_s[:, dt, m * P:(m + 1) * P],
                                    kT_s[:, dt, jb * 512:(jb + 1) * 512],
                                    start=(dt == 0), stop=(dt == DT - 1))
                        for ji in range(2):
                            jb = 2 * jp + ji
                            sl = slice(jb * 512, (jb + 1) * 512)
                            nc.vector.tensor_copy(KTz_s[0:64, 2 * m, sl],
                                                  pss[ji][0:64, :])
                            nc.vector.tensor_copy(KTz_s[64:128, 2 * m + 1, sl],
                                                  pss[ji][64:128, :])

                def emit_vproj(lk):
                    ps = psProj.tile([P, 512], F32, tag="proj", name=f"psv{lk}")
                    for dt in range(DT):
                        nc.tensor.matmul(
                            ps[:, :],
                            vT_s[:, dt, lk * P:(lk + 1) * P],
                            WvT_s[:, dt, :],
                            start=(dt == 0), stop=(dt == DT - 1))
                    nc.vector.tensor_copy(
                        V_s[:, lk, :, 0:DV],
                        ps[:, :].rearrange("p (h e) -> p h e", h=H))

                def emit_attn(h, inline_v=False):
                    mt = h // 2
                    avs = [psProj.tile([DV + 1, 512], F32, tag="av", bufs=2,
                                       name=f"av{h}_{j}") for j in range(NB)]
                    for m in range(LKT):
                        if inline_v:
                            emit_vproj(m)
                        ps = psProj.tile([P, LQ], F32, tag="qk", bufs=2)
                        for jb in range(NB):
                            nc.tensor.matmul(
                                ps[:, jb * 512:(jb + 1) * 512],
                                KTz_s[:, h, m * P:(m + 1) * P],
                                QT_s[:, mt, jb * 512:(jb + 1) * 512],
                                start=True, stop=True)
                        pt = pTp.tile([P, LQ], BF16, tag="pT")
                        nc.scalar.activation(
                            out=pt[:, :], in_=ps[:, :], func=AF.Exp,
                            bias=mb_s[:, h * LKT + m:h * LKT + m + 1],
                            scale=1.0 / 8.0)
                        for jb in range(NB):
                            nc.tensor.matmul(
                                avs[jb][:, :],
                                V_s[:, m, h, :],
                                pt[:, jb * 512:(jb + 1) * 512],
                                start=(m == 0), stop=(m == LKT - 1))
                    po = (h % 2) * DV
                    for jb in range(NB):
                        rc = rcp.tile([1, 512], F32, tag="rc")
                        nc.vector.tensor_copy(rc, avs[jb][DV:DV + 1, :])
                        stg = bcsp.tile([DV, 512], F32, tag="stg")
                        nc.vector.tensor_copy(stg, avs[jb][0:DV, :])
                        rcd = dramp.tile([1, 512], F32, tag="rcd",
                                         name=f"rcd{h}_{jb}")
                        nc.sync.dma_start(out=rcd, in_=rc[0:1, :])
                        bcs = bcsp.tile([DV, 512], F32, tag="bcs")
                        nc.gpsimd.dma_start(out=bcs, in_=_bcast(rcd[0:1, :], DV))
                        nc.vector.reciprocal_approx_fast(out=bcs, in_=bcs)
                        nc.vector.tensor_mul(
                            attnT[po:po + DV, mt, jb * 512:(jb + 1) * 512],
                            stg[:, :], bcs[:, :])
                    if h % 2 == 1:
                        # channel tile kt complete: residual + stat inputs
                        kt = h // 2
                        nc.gpsimd.tensor_add(attnT[:, kt, :], attnT[:, kt, :],
                                             qresT_s[:, kt, :])
                        nc.vector.tensor_copy(xbf[:, kt, :], attnT[:, kt, :])
                        nc.gpsimd.tensor_mul(x2bf[:, kt, :], xbf[:, kt, :],
                                             xbf[:, kt, :])
                        for nh in range(NB):
                            sl = slice(nh * 512, (nh + 1) * 512)
                            sp1 = psProj.tile([1, 512], F32, tag="proj",
                                              name=f"sp1_{kt}_{nh}")
                            nc.tensor.matmul(sp1[:, :], ones_st[:, :],
                                             xbf[:, kt, sl],
                                             start=True, stop=True)
                            nc.vector.tensor_add(s1a[nh], s1a[nh], sp1[:, :])
                            sp2 = psProj.tile([1, 512], F32, tag="proj",
                                              name=f"sp2_{kt}_{nh}")
                            nc.tensor.matmul(sp2[:, :], ones_st[:, :],
                                             x2bf[:, kt, sl],
                                             start=True, stop=True)
                            nc.vector.tensor_add(s2a[nh], s2a[nh], sp2[:, :])

                emit_kproj(0)
                emit_qproj(0)
                for lk in range(LKT):
                    emit_vproj(lk)
                emit_attn(0)
                emit_attn(1)
                for mt in range(1, DT):
                    emit_kproj(mt)
                    emit_qproj(mt)
                    emit_attn(2 * mt)
                    emit_attn(2 * mt + 1)

            # ============ phase C: LN0 -> fc -> LN1 (all ^T) ============
            with (
                tc.tile_pool(name="lnp", bufs=1) as lnp,
                tc.tile_pool(name="chain", bufs=2) as chain,
                tc.tile_pool(name="bcB", bufs=4) as bcB,
                tc.tile_pool(name="wfc", bufs=1) as wfc,
            ):
                LN0bf = lnp.tile([P, DT, LQ], BF16)
                yT = lnp.tile([P, DT, LQ], F32)
                outT = lnp.tile([P, DT, LQ], F32)

                def ln_half(xfull, xb, x2b, g_idx, b_idx, out_f32, out_bf16,
                            nh, label, pre=None):
                    """One 512-query half of a transposed LayerNorm."""
                    sl = slice(nh * 512, (nh + 1) * 512)
                    if pre is not None:
                        s1, s2 = pre[0][nh], pre[1][nh]
                    else:
                        s1 = psProj.tile([1, 512], F32, tag="proj",
                                         name=f"s1{label}{nh}")
                        s2 = psProj.tile([1, 512], F32, tag="proj",
                                         name=f"s2{label}{nh}")
                        for kt in range(DT):
                            nc.tensor.matmul(s1[:, :], ones_st[:, :],
                                             xb[:, kt, sl],
                                             start=(kt == 0),
                                             stop=(kt == DT - 1))
                            nc.tensor.matmul(s2[:, :], ones_st[:, :],
                                             x2b[:, kt, sl],
                                             start=(kt == 0),
                                             stop=(kt == DT - 1))
                    mu = chain.tile([1, 512], F32, tag="mu")
                    nc.vector.tensor_scalar_mul(mu, s1[:, :], 1.0 / D)
                    var = chain.tile([1, 512], F32, tag="var")
                    nc.vector.tensor_mul(var, mu, mu)
                    msq = chain.tile([1, 512], F32, tag="msq")
                    nc.vector.tensor_scalar_mul(msq, s2[:, :], 1.0 / D)
                    nc.vector.tensor_sub(var, msq, var)
                    nc.scalar.activation(out=var, in_=var, func=AF.Sqrt,
                                         bias=eps_t[0:1, :])
                    rstd = chain.tile([1, 512], F32, tag="rstd")
                    nc.vector.reciprocal_approx_fast(out=rstd, in_=var)
                    mrb = chain.tile([1, 2, 512], BF16, tag="mrb")
                    nc.vector.tensor_copy(mrb[:, 0, :], mu[0:1, :])
                    nc.vector.tensor_copy(mrb[:, 1, :], rstd[0:1, :])
                    mu_b = psProj.tile([P, 512], F32, tag="av",
                                       name=f"mb{label}{nh}")
                    nc.tensor.matmul(mu_b[:, :], ones_r1[:, :], mrb[:, 0, :],
                                     start=True, stop=True)
                    rstd_b = psProj.tile([P, 512], F32, tag="av",
                                         name=f"rb{label}{nh}")
                    nc.tensor.matmul(rstd_b[:, :], ones_r1[:, :], mrb[:, 1, :],
                                     start=True, stop=True)
                    for kt in range(DT):
                        nc.vector.tensor_sub(xfull[:, kt, sl], xfull[:, kt, sl],
                                             mu_b[:, :])
                        nc.vector.tensor_mul(xfull[:, kt, sl], xfull[:, kt, sl],
                                             rstd_b[:, :])
                        nc.gpsimd.tensor_scalar(
                            out=out_f32[:, kt, sl], in0=xfull[:, kt, sl],
                            scalar1=gbT[:, g_idx, kt:kt + 1],
                            scalar2=gbT[:, b_idx, kt:kt + 1],
                            op0=Alu.mult, op1=Alu.add)
                        if out_bf16 is not None:
                            nc.scalar.copy(out_bf16[:, kt, sl],
                                           out_f32[:, kt, sl])

                fcwT_s = wfc.tile([P, DT, D], BF16)
                nc.sync.dma_start(out=fcwT_s, in_=fcwT[:, :, :])
                ybf = statin.tile([P, DT, LQ], BF16, tag="xbf")
                y2bf = statin.tile([P, DT, LQ], BF16, tag="x2bf")

                # LN0 both halves first: xbf/x2bf are fully consumed
                # before ybf/y2bf reuse their slots (avoids a slot-reuse
                # ordering cycle); fc(half0) still overlaps LN0(half1)
                # through real dataflow deps.
                for nh in range(NB):
                    ln_half(attnT, xbf, x2bf, 0, 1, attnT, LN0bf, nh, "a",
                            pre=(s1a, s2a))
                for nh in range(NB):
                    sl = slice(nh * 512, (nh + 1) * 512)
                    # fc for this half
                    for m in range(DT):
                        ps = psProj.tile([P, 512], F32, tag="av",
                                      name=f"fc{m}_{nh}")
                        for dt in range(DT):
                            nc.tensor.matmul(
                                ps[:, :],
                                fcwT_s[:, dt, m * P:(m + 1) * P],
                                LN0bf[:, dt, sl],
                                start=(dt == 0), stop=(dt == DT - 1))
                        # y = fc + fc_b (per-partition) + LN0 residual
                        nc.vector.tensor_scalar_add(yT[:, m, sl], ps[:, :],
                                                    gbT[:, 4, m:m + 1])
                        nc.gpsimd.tensor_add(yT[:, m, sl], yT[:, m, sl],
                                             attnT[:, m, sl])
                        nc.scalar.copy(ybf[:, m, sl], yT[:, m, sl])
                        nc.gpsimd.tensor_mul(y2bf[:, m, sl], ybf[:, m, sl],
                                             ybf[:, m, sl])
                    # LN1 on this half
                    ln_half(yT, ybf, y2bf, 2, 3, outT, None, nh, "b")
                    for kt in range(DT):
                        nc.sync.dma_start(out=out[:, kt, sl],
                                          in_=outT[:, kt, sl])


def _build():
    if "nc" in _CACHE:
        return _CACHE["nc"]
    nc = bacc.Bacc(None, target_bir_lowering=False, debug=False)
    with tile.TileContext(nc) as tc:
        _emit(nc, tc)
    nc.compile()
    _CACHE["nc"] = nc
    return nc


def _prep_in_maps(q, k, v, mask, Wq, Wk, Wv, fc_w, fc_b, g0, b0, g1, b1):
    q = np.asarray(q, np.float32)
    k = np.asarray(k, np.float32)
    v = np.asarray(v, np.float32)
    mask = np.asarray(mask)
    bf = mybir.dt.np(BF16)

    def ptile(a):
        # [n, m] -> transpose -> [m(=tiles*128), n] -> [128, tiles, n]
        t = np.asarray(a, np.float32).T
        return np.ascontiguousarray(
            t.reshape(DT, P, t.shape[1]).transpose(1, 0, 2))

    WqTh = ptile(Wq).astype(bf)
    WkTh = ptile(Wk).astype(bf)
    WvTh = ptile(Wv).astype(bf)
    fcwTh = ptile(fc_w).astype(bf)
    vecs = np.stack([np.asarray(x, np.float32).reshape(DT, P).T
                     for x in (g0, b0, g1, b1, fc_b)])
    vecs = np.ascontiguousarray(vecs)

    in_maps = []
    for c in range(NCORES):
        b = c // 2
        r0 = (c % 2) * LQ
        qTb = ptile(q[b][r0:r0 + LQ]).astype(bf)
        kTb = ptile(k[b]).astype(bf)
        vTb = ptile(v[b]).astype(bf)
        qrTb = ptile(q[b][r0:r0 + LQ])
        mbh = np.zeros((P, H, LKT), np.float32)
        for h in range(H):
            mh = mask[h * B + b].reshape(LKT, P).T  # [p, tile]
            mbh[:, h, :] = np.where(mh == 0, np.float32(NEG), np.float32(0.0))
        in_maps.append({
            "qT": qTb, "kT": kTb, "vT": vTb, "qresT": qrTb,
            "WqT": WqTh, "WkT": WkTh, "WvT": WvTh, "fcwT": fcwTh,
            "mb": np.ascontiguousarray(mbh.reshape(P, H * LKT)),
            "vecs": vecs,
        })
    return in_maps


def kernel(q, k, v, mask, Wq, Wk, Wv, fc_w, fc_b, g0, b0, g1, b1):
    in_maps = _prep_in_maps(q, k, v, mask, Wq, Wk, Wv, fc_w, fc_b,
                            g0, b0, g1, b1)
    nc = _build()
    res = run_bass_kernel_spmd(nc, in_maps, core_ids=list(range(NCORES)))
    outf = np.empty((B, L, D), np.float32)
    for c in range(NCORES):
        b = c // 2
        r0 = (c % 2) * LQ
        o = res.results[c]["out"]  # [128, DT, LQ]
        outf[b, r0:r0 + LQ, :] = o.transpose(2, 1, 0).reshape(LQ, D)
    return outf


# revision 34
# speedup vs baseline: 1.4950x; 1.4950x over previous
"""Trainium2 Bass kernel for the MultiHeadAttention transformer block.

Sharding: 8 cores, core c handles batch b=c//2 and query-row half
(c%2)*1024 .. +1024, all 8 heads.  Each core is fully independent
(no collectives).

Layout strategy: everything lives transposed on chip — [channel/d on
partitions, sequence on free dim] — from the input loads through the
final LayerNorm, so no on-chip transposes are ever needed:
  - scores are computed as S^T[lk, lq]; the key mask is a
    per-partition bias folded into the Exp activation,
  - K^T is stored zero-padded to the full 128 contraction rows (even
    heads in rows 0:64, odd heads in rows 64:128, zeros elsewhere) so
    every QK matmul runs with k=128 and full PE-array activity (the
    HAM clock gate watches array utilization),
  - A@V runs with V stationary producing attn^T directly; softmax
    row-sums come from an appended ones-column on V and are applied
    via a DRAM-bounced partition-broadcast of the row reciprocals,
  - LayerNorm mean/var come from ones-column matmuls over the channel
    (partition) dim on bf16 copies; gamma/beta/fc-bias are
    per-partition scalars in this layout; the whole LN0 -> fc -> LN1
    tail is pipelined in two independent 512-query halves,
  - the fc output projection consumes LN0^T directly and produces
    out^T, which the host un-transposes for free.
"""

import sys

if "/opt/trn_rl_repo" not in sys.path:
    sys.path.insert(0, "/opt/trn_rl_repo")

import numpy as np

import concourse.bacc as bacc
import concourse.bass as bass
import concourse.tile as tile
from concourse import mybir
from concourse.bass_utils import run_bass_kernel_spmd

H, D, DK, DV = 8, 512, 64, 64
B, L = 4, 2048
P = 128
LQ = L // 2          # query rows per core
NCORES = 8
EPS = 1e-5
NEG = -1e9 / 8.0     # masked score after the /temperature divide
F32 = mybir.dt.float32
BF16 = mybir.dt.bfloat16
AF = mybir.ActivationFunctionType
Alu = mybir.AluOpType

DT = D // P     # 4 d-tiles
LKT = L // P    # 16 key tiles
NB = LQ // 512  # 2 psum-bank columns of queries

_CACHE = {}


def _bcast(ap, parts):
    """Partition-broadcast view of a [1, n] DRAM AP for DMA replication."""
    return ap.to_broadcast([parts] + list(ap.shape[1:]))


def _emit(nc, tc):
    qT = nc.dram_tensor("qT", [P, DT, LQ], BF16, kind="ExternalInput")
    kT = nc.dram_tensor("kT", [P, DT, L], BF16, kind="ExternalInput")
    vT = nc.dram_tensor("vT", [P, DT, L], BF16, kind="ExternalInput")
    qresT = nc.dram_tensor("qresT", [P, DT, LQ], F32, kind="ExternalInput")
    WqT = nc.dram_tensor("WqT", [P, DT, D], BF16, kind="ExternalInput")
    WkT = nc.dram_tensor("WkT", [P, DT, D], BF16, kind="ExternalInput")
    WvT = nc.dram_tensor("WvT", [P, DT, D], BF16, kind="ExternalInput")
    fcwT = nc.dram_tensor("fcwT", [P, DT, D], BF16, kind="ExternalInput")
    mb = nc.dram_tensor("mb", [P, H * LKT], F32, kind="ExternalInput")
    vecs = nc.dram_tensor("vecs", [5, P, DT], F32, kind="ExternalInput")
    out = nc.dram_tensor("out", [P, DT, LQ], F32, kind="ExternalOutput")

    with (
        tc.tile_pool(name="consts", bufs=1) as consts,
        tc.tile_pool(name="projout", bufs=1) as projout,
        tc.tile_pool(name="dramp", bufs=3, space="DRAM") as dramp,
        tc.tile_pool(name="psA", bufs=2, space="PSUM") as psProj,
    ):
        # ---- constants resident for the whole kernel ----
        mb_s = consts.tile([P, H * LKT], F32)
        nc.sync.dma_start(out=mb_s, in_=mb[:, :])
        gbT = consts.tile([P, 5, DT], F32)   # g0,b0,g1,b1,fcb as [p, dt]
        for i in range(5):
            nc.sync.dma_start(out=gbT[:, i, :], in_=vecs[i, :, :])
        eps_t = consts.tile([P, 1], F32)
        nc.vector.memset(eps_t, EPS)
        ones_st = consts.tile([P, 1], BF16)  # stats reduction column
        nc.vector.memset(ones_st, 1.0)
        ones_r1 = consts.tile([1, P], BF16)  # rank-1 broadcast row
        nc.vector.memset(ones_r1, 1.0)
        warm = consts.tile([P, 512], BF16)   # PE warm-up fodder
        nc.vector.memset(warm[:, :], 0.0)

        # ---- persistent big tiles ----
        QT_s = projout.tile([P, DT, LQ], BF16)         # Q^T  [ch, lq]
        KTz_s = projout.tile([P, H, L], BF16)          # zero-padded K^T
        V_s = projout.tile([P, LKT, H, DV + 1], BF16)  # V rows + ones col
        attnT = projout.tile([P, DT, LQ], F32)         # attention out ^T
        qresT_s = projout.tile([P, DT, LQ], F32)

        with tc.tile_pool(name="statin", bufs=1) as statin:
            xbf = statin.tile([P, DT, LQ], BF16, tag="xbf")
            x2bf = statin.tile([P, DT, LQ], BF16, tag="x2bf")
            s1a = [statin.tile([1, 512], F32, tag=f"s1a{j}", name=f"s1a{j}")
                   for j in range(NB)]
            s2a = [statin.tile([1, 512], F32, tag=f"s2a{j}", name=f"s2a{j}")
                   for j in range(NB)]
            for j in range(NB):
                nc.vector.memset(s1a[j], 0.0)
                nc.vector.memset(s2a[j], 0.0)

            # ====== phases A+B fused: per-pair projections + attention ======
            with (
                tc.tile_pool(name="inp", bufs=1) as inp,
                tc.tile_pool(name="wts", bufs=1) as wts,
                tc.tile_pool(name="pT", bufs=8) as pTp,
                tc.tile_pool(name="rcp", bufs=3) as rcp,
                tc.tile_pool(name="bcsp", bufs=3) as bcsp,
            ):
                for w in range(24):
                    wps = psProj.tile([P, 512], F32, tag="proj",
                                      name=f"warm{w}")
                    nc.tensor.matmul(wps[:, :], warm[:, 0:P], warm[:, :],
                                     start=True, stop=True)
                WqT_s = wts.tile([P, DT, D], BF16)
                qT_s = inp.tile([P, DT, LQ], BF16)
                WkT_s = wts.tile([P, DT, D], BF16)
                kT_s = inp.tile([P, DT, L], BF16)
                WvT_s = wts.tile([P, DT, D], BF16)
                vT_s = inp.tile([P, DT, L], BF16)
                for dt in range(DT):
                    nc.sync.dma_start(out=WkT_s[:, dt, :], in_=WkT[:, dt, :])
                    nc.sync.dma_start(out=kT_s[:, dt, :], in_=kT[:, dt, :])
                for dt in range(DT):
                    nc.sync.dma_start(out=WqT_s[:, dt, :], in_=WqT[:, dt, :])
                    nc.sync.dma_start(out=qT_s[:, dt, :], in_=qT[:, dt, :])
                for dt in range(DT):
                    nc.sync.dma_start(out=WvT_s[:, dt, :], in_=WvT[:, dt, :])
                    nc.sync.dma_start(out=vT_s[:, dt, :], in_=vT[:, dt, :])
                for dt in range(DT):
                    nc.sync.dma_start(out=qresT_s[:, dt, :],
                                      in_=qresT[:, dt, :])

                # zero the pad rows of KTz: even heads live in rows 0:64,
                # odd heads in rows 64:128 — zero the other half.
                nc.vector.memset(KTz_s[64:128, 0:H:2, :], 0.0)
                nc.vector.memset(KTz_s[0:64, 1:H:2, :], 0.0)
                nc.vector.memset(V_s[:, :, :, DV:DV + 1], 1.0)

                def emit_qproj(m):
                    pss = [psProj.tile([P, 512], F32, tag="proj",
                                       name=f"psq{m}_{j}") for j in range(NB)]
                    for dt in range(DT):
                        for jb in range(NB):
                            nc.tensor.matmul(
                                pss[jb][:, :],
                                WqT_s[:, dt, m * P:(m + 1) * P],
                                qT_s[:, dt, jb * 512:(jb + 1) * 512],
                                start=(dt == 0), stop=(dt == DT - 1))
                    for jb in range(NB):
                        nc.vector.tensor_copy(
                            QT_s[:, m, jb * 512:(jb + 1) * 512], pss[jb][:, :])

                def emit_kproj(m):
                    for jp in range(2):
                        pss = [psProj.tile([P, 512], F32, tag="proj",
                                           name=f"psk{m}_{jp}_{j}")
                               for j in range(2)]
                        for dt in range(DT):
                            for ji in range(2):
                                jb = 2 * jp + ji
                                nc.tensor.matmul(
                                    pss[ji][:, :],
                                    WkT_s[:, dt, m * P:(m + 1) * P],
                                    kT_s[:, dt, jb * 512:(jb + 1) * 512],
                                    start=(dt == 0), stop=(dt == DT - 1))
                        for ji in range(2):
                            jb = 2 * jp + ji
                            sl = slice(jb * 512, (jb + 1) * 512)
                            nc.vector.tensor_copy(KTz_s[0:64, 2 * m, sl],
                                                  pss[ji][0:64, :])
                            nc.vector.tensor_copy(KTz_s[64:128, 2 * m + 1, sl],
                                                  pss[ji][64:128, :])

                def emit_vproj(lk):
                    ps = psProj.tile([P, 512], F32, tag="proj", name=f"psv{lk}")
                    for dt in range(DT):
                        nc.tensor.matmul(
                            ps[:, :],
                            vT_s[:, dt, lk * P:(lk + 1) * P],
                            WvT_s[:, dt, :],
                            start=(dt == 0), stop=(dt == DT - 1))
                    nc.vector.tensor_copy(
                        V_s[:, lk, :, 0:DV],
                        ps[:, :].rearrange("p (h e) -> p h e", h=H))

                def emit_attn(h, inline_v=False):
                    mt = h // 2
                    avs = [psProj.tile([DV + 1, 512], F32, tag="av", bufs=2,
                                       name=f"av{h}_{j}") for j in range(NB)]
                    for m in range(LKT):
                        if inline_v:
                            emit_vproj(m)
                        ps = psProj.tile([P, LQ], F32, tag="qk", bufs=2)
                        for jb in range(NB):
                            nc.tensor.matmul(
                                ps[:, jb * 512:(jb + 1) * 512],
                                KTz_s[:, h, m * P:(m + 1) * P],
                                QT_s[:, mt, jb * 512:(jb + 1) * 512],
                                start=True, stop=True)
                        pt = pTp.tile([P, LQ], BF16, tag="pT")
                        nc.scalar.activation(
                            out=pt[:, :], in_=ps[:, :], func=AF.Exp,
                            bias=mb_s[:, h * LKT + m:h * LKT + m + 1],
                            scale=1.0 / 8.0)
                        for jb in range(NB):
                            nc.tensor.matmul(
                                avs[jb][:, :],
                                V_s[:, m, h, :],
                                pt[:, jb * 512:(jb + 1) * 512],
                                start=(m == 0), stop=(m == LKT - 1))
                    po = (h % 2) * DV
                    for jb in range(NB):
                        rc = rcp.tile([1, 512], F32, tag="rc")
                        nc.vector.tensor_copy(rc, avs[jb][DV:DV + 1, :])
                        stg = bcsp.tile([DV, 512], F32, tag="stg")
                        nc.vector.tensor_copy(stg, avs[jb][0:DV, :])
                        rcd = dramp.tile([1, 512], F32, tag="rcd",
                                         name=f"rcd{h}_{jb}")
                        nc.sync.dma_start(out=rcd, in_=rc[0:1, :])
                        bcs = bcsp.tile([DV, 512], F32, tag="bcs")
                        nc.gpsimd.dma_start(out=bcs, in_=_bcast(rcd[0:1, :], DV))
                        nc.vector.reciprocal_approx_fast(out=bcs, in_=bcs)
                        nc.vector.tensor_mul(
                            attnT[po:po + DV, mt, jb * 512:(jb + 1) * 512],
                            stg[:, :], bcs[:, :])
                    if h % 2 == 1:
                        # channel tile kt complete: residual + stat inputs
                        kt = h // 2
                        nc.gpsimd.tensor_add(attnT[:, kt, :], attnT[:, kt, :],
                                             qresT_s[:, kt, :])
                        nc.vector.tensor_copy(xbf[:, kt, :], attnT[:, kt, :])
                        nc.gpsimd.tensor_mul(x2bf[:, kt, :], xbf[:, kt, :],
                                             xbf[:, kt, :])


                emit_kproj(0)
                emit_qproj(0)
                for lk in range(LKT):
                    emit_vproj(lk)
                emit_attn(0)
                emit_attn(1)
                for mt in range(1, DT):
                    emit_kproj(mt)
                    emit_qproj(mt)
                    emit_attn(2 * mt)
                    emit_attn(2 * mt + 1)

            # ============ phase C: LN0 -> fc -> LN1 (all ^T) ============
            with (
                tc.tile_pool(name="lnp", bufs=1) as lnp,
                tc.tile_pool(name="chain", bufs=2) as chain,
                tc.tile_pool(name="bcB", bufs=4) as bcB,
                tc.tile_pool(name="wfc", bufs=1) as wfc,
            ):
                LN0bf = lnp.tile([P, DT, LQ], BF16)
                yT = lnp.tile([P, DT, LQ], F32)
                outT = lnp.tile([P, DT, LQ], F32)

                def ln_half(xfull, xb, x2b, g_idx, b_idx, out_f32, out_bf16,
                            nh, label, pre=None):
                    """One 512-query half of a transposed LayerNorm."""
                    sl = slice(nh * 512, (nh + 1) * 512)
                    if pre is not None:
                        s1, s2 = pre[0][nh], pre[1][nh]
                    else:
                        s1 = psProj.tile([1, 512], F32, tag="proj",
                                         name=f"s1{label}{nh}")
                        s2 = psProj.tile([1, 512], F32, tag="proj",
                                         name=f"s2{label}{nh}")
                        for kt in range(DT):
                            nc.tensor.matmul(s1[:, :], ones_st[:, :],
                                             xb[:, kt, sl],
                                             start=(kt == 0),
                                             stop=(kt == DT - 1))
                            nc.tensor.matmul(s2[:, :], ones_st[:, :],
                                             x2b[:, kt, sl],
                                             start=(kt == 0),
                                             stop=(kt == DT - 1))
                    mu = chain.tile([1, 512], F32, tag="mu")
                    nc.vector.tensor_scalar_mul(mu, s1[:, :], 1.0 / D)
                    var = chain.tile([1, 512], F32, tag="var")
                    nc.vector.tensor_mul(var, mu, mu)
                    msq = chain.tile([1, 512], F32, tag="msq")
                    nc.vector.tensor_scalar_mul(msq, s2[:, :], 1.0 / D)
                    nc.vector.tensor_sub(var, msq, var)
                    nc.scalar.activation(out=var, in_=var, func=AF.Sqrt,
                                         bias=eps_t[0:1, :])
                    rstd = chain.tile([1, 512], F32, tag="rstd")
                    nc.vector.reciprocal_approx_fast(out=rstd, in_=var)
                    mrb = chain.tile([1, 2, 512], BF16, tag="mrb")
                    nc.vector.tensor_copy(mrb[:, 0, :], mu[0:1, :])
                    nc.vector.tensor_copy(mrb[:, 1, :], rstd[0:1, :])
                    mu_b = psProj.tile([P, 512], F32, tag="av",
                                       name=f"mb{label}{nh}")
                    nc.tensor.matmul(mu_b[:, :], ones_r1[:, :], mrb[:, 0, :],
                                     start=True, stop=True)
                    rstd_b = psProj.tile([P, 512], F32, tag="av",
                                         name=f"rb{label}{nh}")
                    nc.tensor.matmul(rstd_b[:, :], ones_r1[:, :], mrb[:, 1, :],
                                     start=True, stop=True)
                    for kt in range(DT):
                        nc.vector.tensor_sub(xfull[:, kt, sl], xfull[:, kt, sl],
                                             mu_b[:, :])
                        nc.vector.tensor_mul(xfull[:, kt, sl], xfull[:, kt, sl],
                                             rstd_b[:, :])
                        nc.gpsimd.tensor_scalar(
                            out=out_f32[:, kt, sl], in0=xfull[:, kt, sl],
                            scalar1=gbT[:, g_idx, kt:kt + 1],
                            scalar2=gbT[:, b_idx, kt:kt + 1],
                            op0=Alu.mult, op1=Alu.add)
                        if out_bf16 is not None:
                            nc.scalar.copy(out_bf16[:, kt, sl],
                                           out_f32[:, kt, sl])

                fcwT_s = wfc.tile([P, DT, D], BF16)
                nc.sync.dma_start(out=fcwT_s, in_=fcwT[:, :, :])
                ybf = statin.tile([P, DT, LQ], BF16, tag="xbf")
                y2bf = statin.tile([P, DT, LQ], BF16, tag="x2bf")

                # LN0 both halves first: xbf/x2bf are fully consumed
                # before ybf/y2bf reuse their slots (avoids a slot-reuse
                # ordering cycle); fc(half0) still overlaps LN0(half1)
                # through real dataflow deps.
                for nh in range(NB):
                    ln_half(attnT, xbf, x2bf, 0, 1, attnT, LN0bf, nh, "a")
                for nh in range(NB):
                    sl = slice(nh * 512, (nh + 1) * 512)
                    # fc for this half
                    for m in range(DT):
                        ps = psProj.tile([P, 512], F32, tag="av",
                                      name=f"fc{m}_{nh}")
                        for dt in range(DT):
                            nc.tensor.matmul(
                                ps[:, :],
                                fcwT_s[:, dt, m * P:(m + 1) * P],
                                LN0bf[:, dt, sl],
                                start=(dt == 0), stop=(dt == DT - 1))
                        # y = fc + fc_b (per-partition) + LN0 residual
                        nc.vector.tensor_scalar_add(yT[:, m, sl], ps[:, :],
                                                    gbT[:, 4, m:m + 1])
                        nc.gpsimd.tensor_add(yT[:, m, sl], yT[:, m, sl],
                                             attnT[:, m, sl])
                        nc.scalar.copy(ybf[:, m, sl], yT[:, m, sl])
                        nc.gpsimd.tensor_mul(y2bf[:, m, sl], ybf[:, m, sl],
                                             ybf[:, m, sl])
                    # LN1 on this half
                    ln_half(yT, ybf, y2bf, 2, 3, outT, None, nh, "b")
                    for kt in range(DT):
                        nc.sync.dma_start(out=out[:, kt, sl],
                                          in_=outT[:, kt, sl])


def _build():
    if "nc" in _CACHE:
        return _CACHE["nc"]
    nc = bacc.Bacc(None, target_bir_lowering=False, debug=False)
    with tile.TileContext(nc) as tc:
        _emit(nc, tc)
    nc.compile()
    _CACHE["nc"] = nc
    return nc


def _prep_in_maps(q, k, v, mask, Wq, Wk, Wv, fc_w, fc_b, g0, b0, g1, b1):
    q = np.asarray(q, np.float32)
    k = np.asarray(k, np.float32)
    v = np.asarray(v, np.float32)
    mask = np.asarray(mask)
    bf = mybir.dt.np(BF16)

    def ptile(a):
        # [n, m] -> transpose -> [m(=tiles*128), n] -> [128, tiles, n]
        t = np.asarray(a, np.float32).T
        return np.ascontiguousarray(
            t.reshape(DT, P, t.shape[1]).transpose(1, 0, 2))

    WqTh = ptile(Wq).astype(bf)
    WkTh = ptile(Wk).astype(bf)
    WvTh = ptile(Wv).astype(bf)
    fcwTh = ptile(fc_w).astype(bf)
    vecs = np.stack([np.asarray(x, np.float32).reshape(DT, P).T
                     for x in (g0, b0, g1, b1, fc_b)])
    vecs = np.ascontiguousarray(vecs)

    in_maps = []
    for c in range(NCORES):
        b = c // 2
        r0 = (c % 2) * LQ
        qTb = ptile(q[b][r0:r0 + LQ]).astype(bf)
        kTb = ptile(k[b]).astype(bf)
        vTb = ptile(v[b]).astype(bf)
        qrTb = ptile(q[b][r0:r0 + LQ])
        mbh = np.zeros((P, H, LKT), np.float32)
        for h in range(H):
            mh = mask[h * B + b].reshape(LKT, P).T  # [p, tile]
            mbh[:, h, :] = np.where(mh == 0, np.float32(NEG), np.float32(0.0))
        in_maps.append({
            "qT": qTb, "kT": kTb, "vT": vTb, "qresT": qrTb,
            "WqT": WqTh, "WkT": WkTh, "WvT": WvTh, "fcwT": fcwTh,
            "mb": np.ascontiguousarray(mbh.reshape(P, H * LKT)),
            "vecs": vecs,
        })
    return in_maps


def kernel(q, k, v, mask, Wq, Wk, Wv, fc_w, fc_b, g0, b0, g1, b1):
    in_maps = _prep_in_maps(q, k, v, mask, Wq, Wk, Wv, fc_w, fc_b,
                            g0, b0, g1, b1)
    nc = _build()
    res = run_bass_kernel_spmd(nc, in_maps, core_ids=list(range(NCORES)))
    outf = np.empty((B, L, D), np.float32)
    for c in range(NCORES):
        b = c // 2
        r0 = (c % 2) * LQ
        o = res.results[c]["out"]  # [128, DT, LQ]
        outf[b, r0:r0 + LQ, :] = o.transpose(2, 1, 0).reshape(LQ, D)
    return outf


# revision 35
# speedup vs baseline: 1.5161x; 1.0141x over previous
"""Trainium2 Bass kernel for the MultiHeadAttention transformer block.

Sharding: 8 cores, core c handles batch b=c//2 and query-row half
(c%2)*1024 .. +1024, all 8 heads.  Each core is fully independent
(no collectives).

Layout strategy: everything lives transposed on chip — [channel/d on
partitions, sequence on free dim] — from the input loads through the
final LayerNorm, so no on-chip transposes are ever needed:
  - scores are computed as S^T[lk, lq]; the key mask is a
    per-partition bias folded into the Exp activation,
  - K^T is stored zero-padded to the full 128 contraction rows (even
    heads in rows 0:64, odd heads in rows 64:128, zeros elsewhere) so
    every QK matmul runs with k=128 and full PE-array activity (the
    HAM clock gate watches array utilization),
  - A@V runs with V stationary producing attn^T directly; softmax
    row-sums come from an appended ones-column on V and are applied
    via a DRAM-bounced partition-broadcast of the row reciprocals,
  - LayerNorm mean/var come from ones-column matmuls over the channel
    (partition) dim on bf16 copies; gamma/beta/fc-bias are
    per-partition scalars in this layout; the whole LN0 -> fc -> LN1
    tail is pipelined in two independent 512-query halves,
  - the fc output projection consumes LN0^T directly and produces
    out^T, which the host un-transposes for free.
"""

import sys

if "/opt/trn_rl_repo" not in sys.path:
    sys.path.insert(0, "/opt/trn_rl_repo")

import numpy as np

import concourse.bacc as bacc
import concourse.bass as bass
import concourse.tile as tile
from concourse import mybir
from concourse.bass_utils import run_bass_kernel_spmd

H, D, DK, DV = 8, 512, 64, 64
B, L = 4, 2048
P = 128
LQ = L // 2          # query rows per core
NCORES = 8
EPS = 1e-5
NEG = -1e9 / 8.0     # masked score after the /temperature divide
F32 = mybir.dt.float32
BF16 = mybir.dt.bfloat16
AF = mybir.ActivationFunctionType
Alu = mybir.AluOpType

DT = D // P     # 4 d-tiles
LKT = L // P    # 16 key tiles
NB = LQ // 512  # 2 psum-bank columns of queries

_CACHE = {}


def _bcast(ap, parts):
    """Partition-broadcast view of a [1, n] DRAM AP for DMA replication."""
    return ap.to_broadcast([parts] + list(ap.shape[1:]))


def _emit(nc, tc):
    qT = nc.dram_tensor("qT", [P, DT, LQ], BF16, kind="ExternalInput")
    kT = nc.dram_tensor("kT", [P, DT, L], BF16, kind="ExternalInput")
    vT = nc.dram_tensor("vT", [P, DT, L], BF16, kind="ExternalInput")
    qresT = nc.dram_tensor("qresT", [P, DT, LQ], F32, kind="ExternalInput")
    WqT = nc.dram_tensor("WqT", [P, DT, D], BF16, kind="ExternalInput")
    WkT = nc.dram_tensor("WkT", [P, DT, D], BF16, kind="ExternalInput")
    WvT = nc.dram_tensor("WvT", [P, DT, D], BF16, kind="ExternalInput")
    fcwT = nc.dram_tensor("fcwT", [P, DT, D], BF16, kind="ExternalInput")
    mb = nc.dram_tensor("mb", [P, H * LKT], F32, kind="ExternalInput")
    vecs = nc.dram_tensor("vecs", [5, P, DT], F32, kind="ExternalInput")
    out = nc.dram_tensor("out", [P, DT, LQ], F32, kind="ExternalOutput")

    with (
        tc.tile_pool(name="consts", bufs=1) as consts,
        tc.tile_pool(name="projout", bufs=1) as projout,
        tc.tile_pool(name="dramp", bufs=3, space="DRAM") as dramp,
        tc.tile_pool(name="psA", bufs=2, space="PSUM") as psProj,
    ):
        # ---- constants resident for the whole kernel ----
        mb_s = consts.tile([P, H * LKT], F32)
        nc.sync.dma_start(out=mb_s, in_=mb[:, :])
        gbT = consts.tile([P, 5, DT], F32)   # g0,b0,g1,b1,fcb as [p, dt]
        for i in range(5):
            nc.sync.dma_start(out=gbT[:, i, :], in_=vecs[i, :, :])
        eps_t = consts.tile([P, 1], F32)
        nc.vector.memset(eps_t, EPS)
        ones_st = consts.tile([P, 1], BF16)  # stats reduction column
        nc.vector.memset(ones_st, 1.0)
        ones_r1 = consts.tile([1, P], BF16)  # rank-1 broadcast row
        nc.vector.memset(ones_r1, 1.0)
        warm = consts.tile([P, 512], BF16)   # PE warm-up fodder
        nc.vector.memset(warm[:, :], 0.0)

        # ---- persistent big tiles ----
        QT_s = projout.tile([P, DT, LQ], BF16)         # Q^T  [ch, lq]
        KTz_s = projout.tile([P, H, L], BF16)          # zero-padded K^T
        V_s = projout.tile([P, LKT, H, DV + 1], BF16)  # V rows + ones col
        attnT = projout.tile([P, DT, LQ], F32)         # attention out ^T
        qresT_s = projout.tile([P, DT, LQ], F32)

        with tc.tile_pool(name="statin", bufs=1) as statin:
            xbf = statin.tile([P, DT, LQ], BF16, tag="xbf")
            x2bf = statin.tile([P, DT, LQ], BF16, tag="x2bf")
            s1a = [statin.tile([1, 512], F32, tag=f"s1a{j}", name=f"s1a{j}")
                   for j in range(NB)]
            s2a = [statin.tile([1, 512], F32, tag=f"s2a{j}", name=f"s2a{j}")
                   for j in range(NB)]
            for j in range(NB):
                nc.vector.memset(s1a[j], 0.0)
                nc.vector.memset(s2a[j], 0.0)

            # ====== phases A+B fused: per-pair projections + attention ======
            with (
                tc.tile_pool(name="inp", bufs=1) as inp,
                tc.tile_pool(name="wts", bufs=1) as wts,
                tc.tile_pool(name="pT", bufs=8) as pTp,
                tc.tile_pool(name="rcp", bufs=3) as rcp,
                tc.tile_pool(name="bcsp", bufs=3) as bcsp,
            ):
                for w in range(24):
                    wps = psProj.tile([P, 512], F32, tag="proj",
                                      name=f"warm{w}")
                    nc.tensor.matmul(wps[:, :], warm[:, 0:P], warm[:, :],
                                     start=True, stop=True)
                WqT_s = wts.tile([P, DT, D], BF16)
                qT_s = inp.tile([P, DT, LQ], BF16)
                WkT_s = wts.tile([P, DT, D], BF16)
                kT_s = inp.tile([P, DT, L], BF16)
                WvT_s = wts.tile([P, DT, D], BF16)
                vT_s = inp.tile([P, DT, L], BF16)
                for dt in range(DT):
                    nc.sync.dma_start(out=WkT_s[:, dt, :], in_=WkT[:, dt, :])
                    nc.sync.dma_start(out=kT_s[:, dt, :], in_=kT[:, dt, :])
                for dt in range(DT):
                    nc.sync.dma_start(out=WqT_s[:, dt, :], in_=WqT[:, dt, :])
                    nc.sync.dma_start(out=qT_s[:, dt, :], in_=qT[:, dt, :])
                for dt in range(DT):
                    nc.sync.dma_start(out=WvT_s[:, dt, :], in_=WvT[:, dt, :])
                    nc.sync.dma_start(out=vT_s[:, dt, :], in_=vT[:, dt, :])
                for dt in range(DT):
                    nc.sync.dma_start(out=qresT_s[:, dt, :],
                                      in_=qresT[:, dt, :])

                # zero the pad rows of KTz: even heads live in rows 0:64,
                # odd heads in rows 64:128 — zero the other half.
                nc.vector.memset(KTz_s[64:128, 0:H:2, :], 0.0)
                nc.vector.memset(KTz_s[0:64, 1:H:2, :], 0.0)
                nc.vector.memset(V_s[:, :, :, DV:DV + 1], 1.0)

                def emit_qproj(m):
                    pss = [psProj.tile([P, 512], F32, tag="proj",
                                       name=f"psq{m}_{j}") for j in range(NB)]
                    for dt in range(DT):
                        for jb in range(NB):
                            nc.tensor.matmul(
                                pss[jb][:, :],
                                WqT_s[:, dt, m * P:(m + 1) * P],
                                qT_s[:, dt, jb * 512:(jb + 1) * 512],
                                start=(dt == 0), stop=(dt == DT - 1))
                    for jb in range(NB):
                        nc.vector.tensor_copy(
                            QT_s[:, m, jb * 512:(jb + 1) * 512], pss[jb][:, :])

                def emit_kproj(m):
                    for jp in range(2):
                        pss = [psProj.tile([P, 512], F32, tag="proj",
                                           name=f"psk{m}_{jp}_{j}")
                               for j in range(2)]
                        for dt in range(DT):
                            for ji in range(2):
                                jb = 2 * jp + ji
                                nc.tensor.matmul(
                                    pss[ji][:, :],
                                    WkT_s[:, dt, m * P:(m + 1) * P],
                                    kT_s[:, dt, jb * 512:(jb + 1) * 512],
                                    start=(dt == 0), stop=(dt == DT - 1))
                        for ji in range(2):
                            jb = 2 * jp + ji
                            sl = slice(jb * 512, (jb + 1) * 512)
                            nc.vector.tensor_copy(KTz_s[0:64, 2 * m, sl],
                                                  pss[ji][0:64, :])
                            nc.vector.tensor_copy(KTz_s[64:128, 2 * m + 1, sl],
                                                  pss[ji][64:128, :])

                def emit_vproj(lk):
                    ps = psProj.tile([P, 512], F32, tag="proj", name=f"psv{lk}")
                    for dt in range(DT):
                        nc.tensor.matmul(
                            ps[:, :],
                            vT_s[:, dt, lk * P:(lk + 1) * P],
                            WvT_s[:, dt, :],
                            start=(dt == 0), stop=(dt == DT - 1))
                    nc.vector.tensor_copy(
                        V_s[:, lk, :, 0:DV],
                        ps[:, :].rearrange("p (h e) -> p h e", h=H))

                def emit_attn(h, inline_v=False):
                    mt = h // 2
                    avs = [psProj.tile([DV + 1, 512], F32, tag="av", bufs=2,
                                       name=f"av{h}_{j}") for j in range(NB)]
                    for m in range(LKT):
                        if inline_v:
                            emit_vproj(m)
                        ps = psProj.tile([P, LQ], F32, tag="qk", bufs=2)
                        for jb in range(NB):
                            nc.tensor.matmul(
                                ps[:, jb * 512:(jb + 1) * 512],
                                KTz_s[:, h, m * P:(m + 1) * P],
                                QT_s[:, mt, jb * 512:(jb + 1) * 512],
                                start=True, stop=True)
                        pt = pTp.tile([P, LQ], BF16, tag="pT")
                        nc.scalar.activation(
                            out=pt[:, :], in_=ps[:, :], func=AF.Exp,
                            bias=mb_s[:, h * LKT + m:h * LKT + m + 1],
                            scale=1.0 / 8.0)
                        for jb in range(NB):
                            nc.tensor.matmul(
                                avs[jb][:, :],
                                V_s[:, m, h, :],
                                pt[:, jb * 512:(jb + 1) * 512],
                                start=(m == 0), stop=(m == LKT - 1))
                    po = (h % 2) * DV
                    for jb in range(NB):
                        rc = rcp.tile([1, 512], F32, tag="rc")
                        nc.vector.tensor_copy(rc, avs[jb][DV:DV + 1, :])
                        stg = bcsp.tile([DV, 512], F32, tag="stg")
                        nc.vector.tensor_copy(stg, avs[jb][0:DV, :])
                        rcd = dramp.tile([1, 512], F32, tag="rcd",
                                         name=f"rcd{h}_{jb}")
                        nc.sync.dma_start(out=rcd, in_=rc[0:1, :])
                        bcs = bcsp.tile([DV, 512], F32, tag="bcs")
                        nc.gpsimd.dma_start(out=bcs, in_=_bcast(rcd[0:1, :], DV))
                        nc.vector.reciprocal_approx_fast(out=bcs, in_=bcs)
                        nc.vector.tensor_mul(
                            attnT[po:po + DV, mt, jb * 512:(jb + 1) * 512],
                            stg[:, :], bcs[:, :])
                    if h % 2 == 1:
                        # channel tile kt complete: residual + stat inputs
                        kt = h // 2
                        nc.gpsimd.tensor_add(attnT[:, kt, :], attnT[:, kt, :],
                                             qresT_s[:, kt, :])
                        nc.vector.tensor_copy(xbf[:, kt, :], attnT[:, kt, :])
                        nc.gpsimd.tensor_mul(x2bf[:, kt, :], xbf[:, kt, :],
                                             xbf[:, kt, :])


                emit_kproj(0)
                emit_qproj(0)
                for lk in range(LKT):
                    emit_vproj(lk)
                emit_attn(0)
                emit_attn(1)
                for mt in range(1, DT):
                    emit_kproj(mt)
                    emit_qproj(mt)
                    emit_attn(2 * mt)
                    emit_attn(2 * mt + 1)

            # ============ phase C: LN0 -> fc -> LN1 (all ^T) ============
            with (
                tc.tile_pool(name="lnp", bufs=1) as lnp,
                tc.tile_pool(name="chain", bufs=2) as chain,
                tc.tile_pool(name="bcB", bufs=4) as bcB,
                tc.tile_pool(name="wfc", bufs=1) as wfc,
            ):
                LN0bf = lnp.tile([P, DT, LQ], BF16)
                yT = lnp.tile([P, DT, LQ], F32)
                outT = lnp.tile([P, DT, LQ], F32)

                def ln_half(xfull, xb, x2b, g_idx, b_idx, out_f32, out_bf16,
                            nh, label, pre=None):
                    """One 512-query half of a transposed LayerNorm."""
                    sl = slice(nh * 512, (nh + 1) * 512)
                    if pre is not None:
                        s1, s2 = pre[0][nh], pre[1][nh]
                    else:
                        s1 = psProj.tile([1, 512], F32, tag="proj",
                                         name=f"s1{label}{nh}")
                        s2 = psProj.tile([1, 512], F32, tag="proj",
                                         name=f"s2{label}{nh}")
                        for kt in range(DT):
                            nc.tensor.matmul(s1[:, :], ones_st[:, :],
                                             xb[:, kt, sl],
                                             start=(kt == 0),
                                             stop=(kt == DT - 1))
                            nc.tensor.matmul(s2[:, :], ones_st[:, :],
                                             x2b[:, kt, sl],
                                             start=(kt == 0),
                                             stop=(kt == DT - 1))
                    mu = chain.tile([1, 512], F32, tag="mu")
                    nc.vector.tensor_scalar_mul(mu, s1[:, :], 1.0 / D)
                    var = chain.tile([1, 512], F32, tag="var")
                    nc.vector.tensor_mul(var, mu, mu)
                    msq = chain.tile([1, 512], F32, tag="msq")
                    nc.vector.tensor_scalar_mul(msq, s2[:, :], 1.0 / D)
                    nc.vector.tensor_sub(var, msq, var)
                    nc.scalar.activation(out=var, in_=var, func=AF.Sqrt,
                                         bias=eps_t[0:1, :])
                    rstd = chain.tile([1, 512], F32, tag="rstd")
                    nc.vector.reciprocal_approx_fast(out=rstd, in_=var)
                    mrb = chain.tile([1, 2, 512], BF16, tag="mrb")
                    nc.vector.tensor_copy(mrb[:, 0, :], mu[0:1, :])
                    nc.vector.tensor_copy(mrb[:, 1, :], rstd[0:1, :])
                    mu_b = psProj.tile([P, 512], F32, tag="av",
                                       name=f"mb{label}{nh}")
                    nc.tensor.matmul(mu_b[:, :], ones_r1[:, :], mrb[:, 0, :],
                                     start=True, stop=True)
                    rstd_b = psProj.tile([P, 512], F32, tag="av",
                                         name=f"rb{label}{nh}")
                    nc.tensor.matmul(rstd_b[:, :], ones_r1[:, :], mrb[:, 1, :],
                                     start=True, stop=True)
                    for kt in range(DT):
                        nc.vector.tensor_sub(xfull[:, kt, sl], xfull[:, kt, sl],
                                             mu_b[:, :])
                        nc.vector.tensor_mul(xfull[:, kt, sl], xfull[:, kt, sl],
                                             rstd_b[:, :])
                        nc.vector.tensor_scalar(
                            out=out_f32[:, kt, sl], in0=xfull[:, kt, sl],
                            scalar1=gbT[:, g_idx, kt:kt + 1],
                            scalar2=gbT[:, b_idx, kt:kt + 1],
                            op0=Alu.mult, op1=Alu.add)
                        if out_bf16 is not None:
                            nc.scalar.copy(out_bf16[:, kt, sl],
                                           out_f32[:, kt, sl])

                fcwT_s = wfc.tile([P, DT, D], BF16)
                nc.sync.dma_start(out=fcwT_s, in_=fcwT[:, :, :])
                ybf = statin.tile([P, DT, LQ], BF16, tag="xbf")
                y2bf = statin.tile([P, DT, LQ], BF16, tag="x2bf")

                # LN0 both halves first: xbf/x2bf are fully consumed
                # before ybf/y2bf reuse their slots (avoids a slot-reuse
                # ordering cycle); fc(half0) still overlaps LN0(half1)
                # through real dataflow deps.
                for nh in range(NB):
                    ln_half(attnT, xbf, x2bf, 0, 1, attnT, LN0bf, nh, "a")
                for nh in range(NB):
                    sl = slice(nh * 512, (nh + 1) * 512)
                    # fc for this half
                    for m in range(DT):
                        ps = psProj.tile([P, 512], F32, tag="av",
                                      name=f"fc{m}_{nh}")
                        for dt in range(DT):
                            nc.tensor.matmul(
                                ps[:, :],
                                fcwT_s[:, dt, m * P:(m + 1) * P],
                                LN0bf[:, dt, sl],
                                start=(dt == 0), stop=(dt == DT - 1))
                        # y = fc + fc_b (per-partition) + LN0 residual
                        nc.vector.tensor_scalar_add(yT[:, m, sl], ps[:, :],
                                                    gbT[:, 4, m:m + 1])
                        nc.gpsimd.tensor_add(yT[:, m, sl], yT[:, m, sl],
                                             attnT[:, m, sl])
                        nc.scalar.copy(ybf[:, m, sl], yT[:, m, sl])
                        nc.vector.tensor_mul(y2bf[:, m, sl], ybf[:, m, sl],
                                             ybf[:, m, sl])
                    # LN1 on this half
                    ln_half(yT, ybf, y2bf, 2, 3, outT, None, nh, "b")
                    for kt in range(DT):
                        nc.sync.dma_start(out=out[:, kt, sl],
                                          in_=outT[:, kt, sl])


def _build():
    if "nc" in _CACHE:
        return _CACHE["nc"]
    nc = bacc.Bacc(None, target_bir_lowering=False, debug=False)
    with tile.TileContext(nc) as tc:
        _emit(nc, tc)
    nc.compile()
    _CACHE["nc"] = nc
    return nc


def _prep_in_maps(q, k, v, mask, Wq, Wk, Wv, fc_w, fc_b, g0, b0, g1, b1):
    q = np.asarray(q, np.float32)
    k = np.asarray(k, np.float32)
    v = np.asarray(v, np.float32)
    mask = np.asarray(mask)
    bf = mybir.dt.np(BF16)

    def ptile(a):
        # [n, m] -> transpose -> [m(=tiles*128), n] -> [128, tiles, n]
        t = np.asarray(a, np.float32).T
        return np.ascontiguousarray(
            t.reshape(DT, P, t.shape[1]).transpose(1, 0, 2))

    WqTh = ptile(Wq).astype(bf)
    WkTh = ptile(Wk).astype(bf)
    WvTh = ptile(Wv).astype(bf)
    fcwTh = ptile(fc_w).astype(bf)
    vecs = np.stack([np.asarray(x, np.float32).reshape(DT, P).T
                     for x in (g0, b0, g1, b1, fc_b)])
    vecs = np.ascontiguousarray(vecs)

    in_maps = []
    for c in range(NCORES):
        b = c // 2
        r0 = (c % 2) * LQ
        qTb = ptile(q[b][r0:r0 + LQ]).astype(bf)
        kTb = ptile(k[b]).astype(bf)
        vTb = ptile(v[b]).astype(bf)
        qrTb = ptile(q[b][r0:r0 + LQ])
        mbh = np.zeros((P, H, LKT), np.float32)
        for h in range(H):
            mh = mask[h * B + b].reshape(LKT, P).T  # [p, tile]
            mbh[:, h, :] = np.where(mh == 0, np.float32(NEG), np.float32(0.0))
        in_maps.append({
            "qT": qTb, "kT": kTb, "vT": vTb, "qresT": qrTb,
            "WqT": WqTh, "WkT": WkTh, "WvT": WvTh, "fcwT": fcwTh,
            "mb": np.ascontiguousarray(mbh.reshape(P, H * LKT)),
            "vecs": vecs,
        })
    return in_maps


def kernel(q, k, v, mask, Wq, Wk, Wv, fc_w, fc_b, g0, b0, g1, b1):
    in_maps = _prep_in_maps(q, k, v, mask, Wq, Wk, Wv, fc_w, fc_b,
                            g0, b0, g1, b1)
    nc = _build()
    res = run_bass_kernel_spmd(nc, in_maps, core_ids=list(range(NCORES)))
    outf = np.empty((B, L, D), np.float32)
    for c in range(NCORES):
        b = c // 2
        r0 = (c % 2) * LQ
        o = res.results[c]["out"]  # [128, DT, LQ]
        outf[b, r0:r0 + LQ, :] = o.transpose(2, 1, 0).reshape(LQ, D)
    return outf


# revision 36
# speedup vs baseline: 1.5460x; 1.0197x over previous
"""Trainium2 Bass kernel for the MultiHeadAttention transformer block.

Sharding: 8 cores, core c handles batch b=c//2 and query-row half
(c%2)*1024 .. +1024, all 8 heads.  Each core is fully independent
(no collectives).

Layout strategy: everything lives transposed on chip — [channel/d on
partitions, sequence on free dim] — from the input loads through the
final LayerNorm, so no on-chip transposes are ever needed:
  - scores are computed as S^T[lk, lq]; the key mask is a
    per-partition bias folded into the Exp activation,
  - K^T is stored zero-padded to the full 128 contraction rows (even
    heads in rows 0:64, odd heads in rows 64:128, zeros elsewhere) so
    every QK matmul runs with k=128 and full PE-array activity (the
    HAM clock gate watches array utilization),
  - A@V runs with V stationary producing attn^T directly; softmax
    row-sums come from an appended ones-column on V and are applied
    via a DRAM-bounced partition-broadcast of the row reciprocals,
  - LayerNorm mean/var come from ones-column matmuls over the channel
    (partition) dim on bf16 copies; gamma/beta/fc-bias are
    per-partition scalars in this layout; the whole LN0 -> fc -> LN1
    tail is pipelined in two independent 512-query halves,
  - the fc output projection consumes LN0^T directly and produces
    out^T, which the host un-transposes for free.
"""

import sys

if "/opt/trn_rl_repo" not in sys.path:
    sys.path.insert(0, "/opt/trn_rl_repo")

import numpy as np

import concourse.bacc as bacc
import concourse.bass as bass
import concourse.tile as tile
from concourse import mybir
from concourse.bass_utils import run_bass_kernel_spmd

H, D, DK, DV = 8, 512, 64, 64
B, L = 4, 2048
P = 128
LQ = L // 2          # query rows per core
NCORES = 8
EPS = 1e-5
NEG = -1e9 / 8.0     # masked score after the /temperature divide
F32 = mybir.dt.float32
BF16 = mybir.dt.bfloat16
AF = mybir.ActivationFunctionType
Alu = mybir.AluOpType

DT = D // P     # 4 d-tiles
LKT = L // P    # 16 key tiles
NB = LQ // 512  # 2 psum-bank columns of queries

_CACHE = {}


def _bcast(ap, parts):
    """Partition-broadcast view of a [1, n] DRAM AP for DMA replication."""
    return ap.to_broadcast([parts] + list(ap.shape[1:]))


def _emit(nc, tc):
    qT = nc.dram_tensor("qT", [P, DT, LQ], BF16, kind="ExternalInput")
    kT = nc.dram_tensor("kT", [P, DT, L], BF16, kind="ExternalInput")
    vT = nc.dram_tensor("vT", [P, DT, L], BF16, kind="ExternalInput")
    qresT = nc.dram_tensor("qresT", [P, DT, LQ], BF16, kind="ExternalInput")
    WqT = nc.dram_tensor("WqT", [P, DT, D], BF16, kind="ExternalInput")
    WkT = nc.dram_tensor("WkT", [P, DT, D], BF16, kind="ExternalInput")
    WvT = nc.dram_tensor("WvT", [P, DT, D], BF16, kind="ExternalInput")
    fcwT = nc.dram_tensor("fcwT", [P, DT, D], BF16, kind="ExternalInput")
    mb = nc.dram_tensor("mb", [P, H * LKT], F32, kind="ExternalInput")
    vecs = nc.dram_tensor("vecs", [5, P, DT], F32, kind="ExternalInput")
    out = nc.dram_tensor("out", [P, DT, LQ], F32, kind="ExternalOutput")

    with (
        tc.tile_pool(name="consts", bufs=1) as consts,
        tc.tile_pool(name="projout", bufs=1) as projout,
        tc.tile_pool(name="dramp", bufs=3, space="DRAM") as dramp,
        tc.tile_pool(name="psA", bufs=2, space="PSUM") as psProj,
    ):
        # ---- constants resident for the whole kernel ----
        mb_s = consts.tile([P, H * LKT], F32)
        nc.sync.dma_start(out=mb_s, in_=mb[:, :])
        gbT = consts.tile([P, 5, DT], F32)   # g0,b0,g1,b1,fcb as [p, dt]
        for i in range(5):
            nc.sync.dma_start(out=gbT[:, i, :], in_=vecs[i, :, :])
        eps_t = consts.tile([P, 1], F32)
        nc.vector.memset(eps_t, EPS)
        ones_st = consts.tile([P, 1], BF16)  # stats reduction column
        nc.vector.memset(ones_st, 1.0)
        ones_r1 = consts.tile([1, P], BF16)  # rank-1 broadcast row
        nc.vector.memset(ones_r1, 1.0)
        warm = consts.tile([P, 512], BF16)   # PE warm-up fodder
        nc.vector.memset(warm[:, :], 0.0)

        # ---- persistent big tiles ----
        QT_s = projout.tile([P, DT, LQ], BF16)         # Q^T  [ch, lq]
        KTz_s = projout.tile([P, H, L], BF16)          # zero-padded K^T
        V_s = projout.tile([P, LKT, H, DV + 1], BF16)  # V rows + ones col
        qresT_s = projout.tile([P, DT, LQ], BF16)

        with tc.tile_pool(name="statin", bufs=1) as statin:
            xbf = statin.tile([P, DT, LQ], BF16, tag="xbf")
            x2bf = statin.tile([P, DT, LQ], BF16, tag="x2bf")
            s1a = [statin.tile([1, 512], F32, tag=f"s1a{j}", name=f"s1a{j}")
                   for j in range(NB)]
            s2a = [statin.tile([1, 512], F32, tag=f"s2a{j}", name=f"s2a{j}")
                   for j in range(NB)]
            for j in range(NB):
                nc.vector.memset(s1a[j], 0.0)
                nc.vector.memset(s2a[j], 0.0)

            # ====== phases A+B fused: per-pair projections + attention ======
            with (
                tc.tile_pool(name="inp", bufs=1) as inp,
                tc.tile_pool(name="wts", bufs=1) as wts,
                tc.tile_pool(name="pT", bufs=8) as pTp,
                tc.tile_pool(name="rcp", bufs=3) as rcp,
                tc.tile_pool(name="bcsp", bufs=3) as bcsp,
            ):
                for w in range(24):
                    wps = psProj.tile([P, 512], F32, tag="proj",
                                      name=f"warm{w}")
                    nc.tensor.matmul(wps[:, :], warm[:, 0:P], warm[:, :],
                                     start=True, stop=True)
                WqT_s = wts.tile([P, DT, D], BF16)
                qT_s = inp.tile([P, DT, LQ], BF16)
                WkT_s = wts.tile([P, DT, D], BF16)
                kT_s = inp.tile([P, DT, L], BF16)
                WvT_s = wts.tile([P, DT, D], BF16)
                vT_s = inp.tile([P, DT, L], BF16)
                for dt in range(DT):
                    nc.sync.dma_start(out=WkT_s[:, dt, :], in_=WkT[:, dt, :])
                    nc.sync.dma_start(out=kT_s[:, dt, :], in_=kT[:, dt, :])
                for dt in range(DT):
                    nc.sync.dma_start(out=WqT_s[:, dt, :], in_=WqT[:, dt, :])
                    nc.sync.dma_start(out=qT_s[:, dt, :], in_=qT[:, dt, :])
                for dt in range(DT):
                    nc.sync.dma_start(out=WvT_s[:, dt, :], in_=WvT[:, dt, :])
                    nc.sync.dma_start(out=vT_s[:, dt, :], in_=vT[:, dt, :])
                for dt in range(DT):
                    nc.sync.dma_start(out=qresT_s[:, dt, :],
                                      in_=qresT[:, dt, :])

                # zero the pad rows of KTz: even heads live in rows 0:64,
                # odd heads in rows 64:128 — zero the other half.
                nc.vector.memset(KTz_s[64:128, 0:H:2, :], 0.0)
                nc.vector.memset(KTz_s[0:64, 1:H:2, :], 0.0)
                nc.vector.memset(V_s[:, :, :, DV:DV + 1], 1.0)

                def emit_qproj(m):
                    pss = [psProj.tile([P, 512], F32, tag="proj",
                                       name=f"psq{m}_{j}") for j in range(NB)]
                    for dt in range(DT):
                        for jb in range(NB):
                            nc.tensor.matmul(
                                pss[jb][:, :],
                                WqT_s[:, dt, m * P:(m + 1) * P],
                                qT_s[:, dt, jb * 512:(jb + 1) * 512],
                                start=(dt == 0), stop=(dt == DT - 1))
                    for jb in range(NB):
                        nc.vector.tensor_copy(
                            QT_s[:, m, jb * 512:(jb + 1) * 512], pss[jb][:, :])

                def emit_kproj(m):
                    for jp in range(2):
                        pss = [psProj.tile([P, 512], F32, tag="proj",
                                           name=f"psk{m}_{jp}_{j}")
                               for j in range(2)]
                        for dt in range(DT):
                            for ji in range(2):
                                jb = 2 * jp + ji
                                nc.tensor.matmul(
                                    pss[ji][:, :],
                                    WkT_s[:, dt, m * P:(m + 1) * P],
                                    kT_s[:, dt, jb * 512:(jb + 1) * 512],
                                    start=(dt == 0), stop=(dt == DT - 1))
                        for ji in range(2):
                            jb = 2 * jp + ji
                            sl = slice(jb * 512, (jb + 1) * 512)
                            nc.vector.tensor_copy(KTz_s[0:64, 2 * m, sl],
                                                  pss[ji][0:64, :])
                            nc.vector.tensor_copy(KTz_s[64:128, 2 * m + 1, sl],
                                                  pss[ji][64:128, :])

                def emit_vproj(lk):
                    ps = psProj.tile([P, 512], F32, tag="proj", name=f"psv{lk}")
                    for dt in range(DT):
                        nc.tensor.matmul(
                            ps[:, :],
                            vT_s[:, dt, lk * P:(lk + 1) * P],
                            WvT_s[:, dt, :],
                            start=(dt == 0), stop=(dt == DT - 1))
                    nc.vector.tensor_copy(
                        V_s[:, lk, :, 0:DV],
                        ps[:, :].rearrange("p (h e) -> p h e", h=H))

                def emit_attn(h, inline_v=False):
                    mt = h // 2
                    avs = [psProj.tile([DV + 1, 512], F32, tag="av", bufs=2,
                                       name=f"av{h}_{j}") for j in range(NB)]
                    for m in range(LKT):
                        if inline_v:
                            emit_vproj(m)
                        ps = psProj.tile([P, LQ], F32, tag="qk", bufs=2)
                        for jb in range(NB):
                            nc.tensor.matmul(
                                ps[:, jb * 512:(jb + 1) * 512],
                                KTz_s[:, h, m * P:(m + 1) * P],
                                QT_s[:, mt, jb * 512:(jb + 1) * 512],
                                start=True, stop=True)
                        pt = pTp.tile([P, LQ], BF16, tag="pT")
                        nc.scalar.activation(
                            out=pt[:, :], in_=ps[:, :], func=AF.Exp,
                            bias=mb_s[:, h * LKT + m:h * LKT + m + 1],
                            scale=1.0 / 8.0)
                        for jb in range(NB):
                            nc.tensor.matmul(
                                avs[jb][:, :],
                                V_s[:, m, h, :],
                                pt[:, jb * 512:(jb + 1) * 512],
                                start=(m == 0), stop=(m == LKT - 1))
                    po = (h % 2) * DV
                    for jb in range(NB):
                        rc = rcp.tile([1, 512], F32, tag="rc")
                        nc.vector.tensor_copy(rc, avs[jb][DV:DV + 1, :])
                        stg = bcsp.tile([DV, 512], F32, tag="stg")
                        nc.vector.tensor_copy(stg, avs[jb][0:DV, :])
                        rcd = dramp.tile([1, 512], F32, tag="rcd",
                                         name=f"rcd{h}_{jb}")
                        nc.sync.dma_start(out=rcd, in_=rc[0:1, :])
                        bcs = bcsp.tile([DV, 512], F32, tag="bcs")
                        nc.gpsimd.dma_start(out=bcs, in_=_bcast(rcd[0:1, :], DV))
                        nc.vector.reciprocal_approx_fast(out=bcs, in_=bcs)
                        nc.vector.tensor_mul(
                            xbf[po:po + DV, mt, jb * 512:(jb + 1) * 512],
                            stg[:, :], bcs[:, :])
                    if h % 2 == 1:
                        # channel tile kt complete: residual + stat inputs
                        kt = h // 2
                        nc.gpsimd.tensor_add(xbf[:, kt, :], xbf[:, kt, :],
                                             qresT_s[:, kt, :])
                        nc.gpsimd.tensor_mul(x2bf[:, kt, :], xbf[:, kt, :],
                                             xbf[:, kt, :])


                emit_kproj(0)
                emit_qproj(0)
                for lk in range(LKT):
                    emit_vproj(lk)
                emit_attn(0)
                emit_attn(1)
                for mt in range(1, DT):
                    emit_kproj(mt)
                    emit_qproj(mt)
                    emit_attn(2 * mt)
                    emit_attn(2 * mt + 1)

            # ============ phase C: LN0 -> fc -> LN1 (all ^T, bf16) ========
            with (
                tc.tile_pool(name="lnp", bufs=1) as lnp,
                tc.tile_pool(name="chain", bufs=2) as chain,
                tc.tile_pool(name="bcB", bufs=4) as bcB,
                tc.tile_pool(name="wfc", bufs=1) as wfc,
            ):
                outT = lnp.tile([P, DT, LQ], F32)
                y2bf = lnp.tile([P, DT, LQ], BF16)

                def ln_half(xb, x2b, g_idx, b_idx, nh, label, final_out=None):
                    """One 512-query half of a transposed LayerNorm, applied
                    in place on the bf16 tile xb."""
                    sl = slice(nh * 512, (nh + 1) * 512)
                    s1 = psProj.tile([1, 512], F32, tag="proj",
                                     name=f"s1{label}{nh}")
                    s2 = psProj.tile([1, 512], F32, tag="proj",
                                     name=f"s2{label}{nh}")
                    for kt in range(DT):
                        nc.tensor.matmul(s1[:, :], ones_st[:, :],
                                         xb[:, kt, sl],
                                         start=(kt == 0), stop=(kt == DT - 1))
                        nc.tensor.matmul(s2[:, :], ones_st[:, :],
                                         x2b[:, kt, sl],
                                         start=(kt == 0), stop=(kt == DT - 1))
                    mu = chain.tile([1, 512], F32, tag="mu")
                    nc.vector.tensor_scalar_mul(mu, s1[:, :], 1.0 / D)
                    var = chain.tile([1, 512], F32, tag="var")
                    nc.vector.tensor_mul(var, mu, mu)
                    msq = chain.tile([1, 512], F32, tag="msq")
                    nc.vector.tensor_scalar_mul(msq, s2[:, :], 1.0 / D)
                    nc.vector.tensor_sub(var, msq, var)
                    nc.scalar.activation(out=var, in_=var, func=AF.Sqrt,
                                         bias=eps_t[0:1, :])
                    rstd = chain.tile([1, 512], F32, tag="rstd")
                    nc.vector.reciprocal_approx_fast(out=rstd, in_=var)
                    mrb = chain.tile([1, 2, 512], BF16, tag="mrb")
                    nc.vector.tensor_copy(mrb[:, 0, :], mu[0:1, :])
                    nc.vector.tensor_copy(mrb[:, 1, :], rstd[0:1, :])
                    mu_b = psProj.tile([P, 512], F32, tag="av",
                                       name=f"mb{label}{nh}")
                    nc.tensor.matmul(mu_b[:, :], ones_r1[:, :], mrb[:, 0, :],
                                     start=True, stop=True)
                    rstd_b = psProj.tile([P, 512], F32, tag="av",
                                         name=f"rb{label}{nh}")
                    nc.tensor.matmul(rstd_b[:, :], ones_r1[:, :], mrb[:, 1, :],
                                     start=True, stop=True)
                    mu_bb = bcB.tile([P, 512], BF16, tag="bc",
                                     name=f"mbb{label}{nh}")
                    nc.vector.tensor_copy(mu_bb, mu_b[:, :])
                    rstd_bb = bcB.tile([P, 512], BF16, tag="bc",
                                       name=f"rbb{label}{nh}")
                    nc.vector.tensor_copy(rstd_bb, rstd_b[:, :])
                    for kt in range(DT):
                        nc.vector.tensor_sub(xb[:, kt, sl], xb[:, kt, sl],
                                             mu_bb[:, :])
                        nc.vector.tensor_mul(xb[:, kt, sl], xb[:, kt, sl],
                                             rstd_bb[:, :])
                        if final_out is not None:
                            nc.vector.tensor_scalar(
                                out=final_out[:, kt, sl], in0=xb[:, kt, sl],
                                scalar1=gbT[:, g_idx, kt:kt + 1],
                                scalar2=gbT[:, b_idx, kt:kt + 1],
                                op0=Alu.mult, op1=Alu.add)
                        else:
                            nc.vector.tensor_scalar(
                                out=xb[:, kt, sl], in0=xb[:, kt, sl],
                                scalar1=gbT[:, g_idx, kt:kt + 1],
                                scalar2=gbT[:, b_idx, kt:kt + 1],
                                op0=Alu.mult, op1=Alu.add)

                fcwT_s = wfc.tile([P, DT, D], BF16)
                nc.sync.dma_start(out=fcwT_s, in_=fcwT[:, :, :])
                ybf = statin.tile([P, DT, LQ], BF16, tag="x2bf")

                # LN0 both halves (in place on xbf = LN0 output, bf16)
                for nh in range(NB):
                    ln_half(xbf, x2bf, 0, 1, nh, "a")
                for nh in range(NB):
                    sl = slice(nh * 512, (nh + 1) * 512)
                    # fc for this half; y = fc + fc_b + LN0 residual (bf16)
                    for m in range(DT):
                        ps = psProj.tile([P, 512], F32, tag="av",
                                         name=f"fc{m}_{nh}")
                        for dt in range(DT):
                            nc.tensor.matmul(
                                ps[:, :],
                                fcwT_s[:, dt, m * P:(m + 1) * P],
                                xbf[:, dt, sl],
                                start=(dt == 0), stop=(dt == DT - 1))
                        nc.vector.tensor_scalar_add(ybf[:, m, sl], ps[:, :],
                                                    gbT[:, 4, m:m + 1])
                        nc.gpsimd.tensor_add(ybf[:, m, sl], ybf[:, m, sl],
                                             xbf[:, m, sl])
                        nc.vector.tensor_mul(y2bf[:, m, sl], ybf[:, m, sl],
                                             ybf[:, m, sl])
                    # LN1 on this half
                    ln_half(ybf, y2bf, 2, 3, nh, "b", final_out=outT)
                    for kt in range(DT):
                        nc.sync.dma_start(out=out[:, kt, sl],
                                          in_=outT[:, kt, sl])


def _build():
    if "nc" in _CACHE:
        return _CACHE["nc"]
    nc = bacc.Bacc(None, target_bir_lowering=False, debug=False)
    with tile.TileContext(nc) as tc:
        _emit(nc, tc)
    nc.compile()
    _CACHE["nc"] = nc
    return nc


def _prep_in_maps(q, k, v, mask, Wq, Wk, Wv, fc_w, fc_b, g0, b0, g1, b1):
    q = np.asarray(q, np.float32)
    k = np.asarray(k, np.float32)
    v = np.asarray(v, np.float32)
    mask = np.asarray(mask)
    bf = mybir.dt.np(BF16)

    def ptile(a):
        # [n, m] -> transpose -> [m(=tiles*128), n] -> [128, tiles, n]
        t = np.asarray(a, np.float32).T
        return np.ascontiguousarray(
            t.reshape(DT, P, t.shape[1]).transpose(1, 0, 2))

    WqTh = ptile(Wq).astype(bf)
    WkTh = ptile(Wk).astype(bf)
    WvTh = ptile(Wv).astype(bf)
    fcwTh = ptile(fc_w).astype(bf)
    vecs = np.stack([np.asarray(x, np.float32).reshape(DT, P).T
                     for x in (g0, b0, g1, b1, fc_b)])
    vecs = np.ascontiguousarray(vecs)

    in_maps = []
    for c in range(NCORES):
        b = c // 2
        r0 = (c % 2) * LQ
        qTb = ptile(q[b][r0:r0 + LQ]).astype(bf)
        kTb = ptile(k[b]).astype(bf)
        vTb = ptile(v[b]).astype(bf)
        qrTb = ptile(q[b][r0:r0 + LQ]).astype(bf)
        mbh = np.zeros((P, H, LKT), np.float32)
        for h in range(H):
            mh = mask[h * B + b].reshape(LKT, P).T  # [p, tile]
            mbh[:, h, :] = np.where(mh == 0, np.float32(NEG), np.float32(0.0))
        in_maps.append({
            "qT": qTb, "kT": kTb, "vT": vTb, "qresT": qrTb,
            "WqT": WqTh, "WkT": WkTh, "WvT": WvTh, "fcwT": fcwTh,
            "mb": np.ascontiguousarray(mbh.reshape(P, H * LKT)),
            "vecs": vecs,
        })
    return in_maps


def kernel(q, k, v, mask, Wq, Wk, Wv, fc_w, fc_b, g0, b0, g1, b1):
    in_maps = _prep_in_maps(q, k, v, mask, Wq, Wk, Wv, fc_w, fc_b,
                            g0, b0, g1, b1)
    nc = _build()
    res = run_bass_kernel_spmd(nc, in_maps, core_ids=list(range(NCORES)))
    outf = np.empty((B, L, D), np.float32)
    for c in range(NCORES):
        b = c // 2
        r0 = (c % 2) * LQ
        o = res.results[c]["out"]  # [128, DT, LQ]
        outf[b, r0:r0 + LQ, :] = o.transpose(2, 1, 0).reshape(LQ, D)
    return outf


# revision 37
# speedup vs baseline: 1.5495x; 1.0023x over previous
"""Trainium2 Bass kernel for the MultiHeadAttention transformer block.

Sharding: 8 cores, core c handles batch b=c//2 and query-row half
(c%2)*1024 .. +1024, all 8 heads.  Each core is fully independent
(no collectives).

Layout strategy: everything lives transposed on chip — [channel/d on
partitions, sequence on free dim] — from the input loads through the
final LayerNorm, so no on-chip transposes are ever needed:
  - scores are computed as S^T[lk, lq]; the key mask is a
    per-partition bias folded into the Exp activation,
  - K^T is stored zero-padded to the full 128 contraction rows (even
    heads in rows 0:64, odd heads in rows 64:128, zeros elsewhere) so
    every QK matmul runs with k=128 and full PE-array activity (the
    HAM clock gate watches array utilization),
  - A@V runs with V stationary producing attn^T directly; softmax
    row-sums come from an appended ones-column on V and are applied
    via a DRAM-bounced partition-broadcast of the row reciprocals,
  - LayerNorm mean/var come from ones-column matmuls over the channel
    (partition) dim on bf16 copies; gamma/beta/fc-bias are
    per-partition scalars in this layout; the whole LN0 -> fc -> LN1
    tail is pipelined in two independent 512-query halves,
  - the fc output projection consumes LN0^T directly and produces
    out^T, which the host un-transposes for free.
"""

import sys

if "/opt/trn_rl_repo" not in sys.path:
    sys.path.insert(0, "/opt/trn_rl_repo")

import numpy as np

import concourse.bacc as bacc
import concourse.bass as bass
import concourse.tile as tile
from concourse import mybir
from concourse.bass_utils import run_bass_kernel_spmd

H, D, DK, DV = 8, 512, 64, 64
B, L = 4, 2048
P = 128
LQ = L // 2          # query rows per core
NCORES = 8
EPS = 1e-5
NEG = -1e9 / 8.0     # masked score after the /temperature divide
F32 = mybir.dt.float32
BF16 = mybir.dt.bfloat16
AF = mybir.ActivationFunctionType
Alu = mybir.AluOpType

DT = D // P     # 4 d-tiles
LKT = L // P    # 16 key tiles
NB = LQ // 512  # 2 psum-bank columns of queries

_CACHE = {}


def _bcast(ap, parts):
    """Partition-broadcast view of a [1, n] DRAM AP for DMA replication."""
    return ap.to_broadcast([parts] + list(ap.shape[1:]))


def _emit(nc, tc):
    qT = nc.dram_tensor("qT", [P, DT, LQ], BF16, kind="ExternalInput")
    kT = nc.dram_tensor("kT", [P, DT, L], BF16, kind="ExternalInput")
    vT = nc.dram_tensor("vT", [P, DT, L], BF16, kind="ExternalInput")
    qresT = nc.dram_tensor("qresT", [P, DT, LQ], BF16, kind="ExternalInput")
    WqT = nc.dram_tensor("WqT", [P, DT, D], BF16, kind="ExternalInput")
    WkT = nc.dram_tensor("WkT", [P, DT, D], BF16, kind="ExternalInput")
    WvT = nc.dram_tensor("WvT", [P, DT, D], BF16, kind="ExternalInput")
    fcwT = nc.dram_tensor("fcwT", [P, DT, D], BF16, kind="ExternalInput")
    mb = nc.dram_tensor("mb", [P, H * LKT], F32, kind="ExternalInput")
    vecs = nc.dram_tensor("vecs", [5, P, DT], F32, kind="ExternalInput")
    out = nc.dram_tensor("out", [P, DT, LQ], F32, kind="ExternalOutput")

    with (
        tc.tile_pool(name="consts", bufs=1) as consts,
        tc.tile_pool(name="projout", bufs=1) as projout,
        tc.tile_pool(name="dramp", bufs=3, space="DRAM") as dramp,
        tc.tile_pool(name="psA", bufs=2, space="PSUM") as psProj,
    ):
        # ---- constants resident for the whole kernel ----
        mb_s = consts.tile([P, H * LKT], F32)
        nc.sync.dma_start(out=mb_s, in_=mb[:, :])
        gbT = consts.tile([P, 5, DT], F32)   # g0,b0,g1,b1,fcb as [p, dt]
        for i in range(5):
            nc.sync.dma_start(out=gbT[:, i, :], in_=vecs[i, :, :])
        eps_t = consts.tile([P, 1], F32)
        nc.vector.memset(eps_t, EPS)
        ones_st = consts.tile([P, 1], BF16)  # stats reduction column
        nc.vector.memset(ones_st, 1.0)
        ones_r1 = consts.tile([1, P], BF16)  # rank-1 broadcast row
        nc.vector.memset(ones_r1, 1.0)
        warm = consts.tile([P, 512], BF16)   # PE warm-up fodder
        nc.vector.memset(warm[:, :], 0.0)

        # ---- persistent big tiles ----
        QT_s = projout.tile([P, DT, LQ], BF16)         # Q^T  [ch, lq]
        KTz_s = projout.tile([P, H, L], BF16)          # zero-padded K^T
        V_s = projout.tile([P, LKT, H, DV + 1], BF16)  # V rows + ones col
        qresT_s = projout.tile([P, DT, LQ], BF16)

        with tc.tile_pool(name="statin", bufs=1) as statin:
            xbf = statin.tile([P, DT, LQ], BF16, tag="xbf")
            x2bf = statin.tile([P, DT, LQ], BF16, tag="x2bf")
            s1a = [statin.tile([1, 512], F32, tag=f"s1a{j}", name=f"s1a{j}")
                   for j in range(NB)]
            s2a = [statin.tile([1, 512], F32, tag=f"s2a{j}", name=f"s2a{j}")
                   for j in range(NB)]
            for j in range(NB):
                nc.vector.memset(s1a[j], 0.0)
                nc.vector.memset(s2a[j], 0.0)

            # ====== phases A+B fused: per-pair projections + attention ======
            with (
                tc.tile_pool(name="inp", bufs=1) as inp,
                tc.tile_pool(name="wts", bufs=1) as wts,
                tc.tile_pool(name="pT", bufs=8) as pTp,
                tc.tile_pool(name="rcp", bufs=3) as rcp,
                tc.tile_pool(name="bcsp", bufs=3) as bcsp,
            ):
                for w in range(24):
                    wps = psProj.tile([P, 512], F32, tag="proj",
                                      name=f"warm{w}")
                    nc.tensor.matmul(wps[:, :], warm[:, 0:P], warm[:, :],
                                     start=True, stop=True)
                WqT_s = wts.tile([P, DT, D], BF16)
                qT_s = inp.tile([P, DT, LQ], BF16)
                WkT_s = wts.tile([P, DT, D], BF16)
                kT_s = inp.tile([P, DT, L], BF16)
                WvT_s = wts.tile([P, DT, D], BF16)
                vT_s = inp.tile([P, DT, L], BF16)
                for dt in range(DT):
                    nc.sync.dma_start(out=WkT_s[:, dt, :], in_=WkT[:, dt, :])
                    nc.sync.dma_start(out=kT_s[:, dt, :], in_=kT[:, dt, :])
                for dt in range(DT):
                    nc.sync.dma_start(out=WqT_s[:, dt, :], in_=WqT[:, dt, :])
                    nc.sync.dma_start(out=qT_s[:, dt, :], in_=qT[:, dt, :])
                for dt in range(DT):
                    nc.sync.dma_start(out=WvT_s[:, dt, :], in_=WvT[:, dt, :])
                    nc.sync.dma_start(out=vT_s[:, dt, :], in_=vT[:, dt, :])
                for dt in range(DT):
                    nc.sync.dma_start(out=qresT_s[:, dt, :],
                                      in_=qresT[:, dt, :])

                # zero the pad rows of KTz: even heads live in rows 0:64,
                # odd heads in rows 64:128 — zero the other half.
                nc.vector.memset(KTz_s[64:128, 0:H:2, :], 0.0)
                nc.vector.memset(KTz_s[0:64, 1:H:2, :], 0.0)
                nc.vector.memset(V_s[:, :, :, DV:DV + 1], 1.0)

                def emit_qproj(m):
                    pss = [psProj.tile([P, 512], F32, tag="proj",
                                       name=f"psq{m}_{j}") for j in range(NB)]
                    for dt in range(DT):
                        for jb in range(NB):
                            nc.tensor.matmul(
                                pss[jb][:, :],
                                WqT_s[:, dt, m * P:(m + 1) * P],
                                qT_s[:, dt, jb * 512:(jb + 1) * 512],
                                start=(dt == 0), stop=(dt == DT - 1))
                    for jb in range(NB):
                        nc.vector.tensor_copy(
                            QT_s[:, m, jb * 512:(jb + 1) * 512], pss[jb][:, :])

                def emit_kproj(m):
                    for jp in range(2):
                        pss = [psProj.tile([P, 512], F32, tag="proj",
                                           name=f"psk{m}_{jp}_{j}")
                               for j in range(2)]
                        for dt in range(DT):
                            for ji in range(2):
                                jb = 2 * jp + ji
                                nc.tensor.matmul(
                                    pss[ji][:, :],
                                    WkT_s[:, dt, m * P:(m + 1) * P],
                                    kT_s[:, dt, jb * 512:(jb + 1) * 512],
                                    start=(dt == 0), stop=(dt == DT - 1))
                        for ji in range(2):
                            jb = 2 * jp + ji
                            sl = slice(jb * 512, (jb + 1) * 512)
                            nc.vector.tensor_copy(KTz_s[0:64, 2 * m, sl],
                                                  pss[ji][0:64, :])
                            nc.vector.tensor_copy(KTz_s[64:128, 2 * m + 1, sl],
                                                  pss[ji][64:128, :])

                def emit_vproj(lk):
                    ps = psProj.tile([P, 512], F32, tag="proj", name=f"psv{lk}")
                    for dt in range(DT):
                        nc.tensor.matmul(
                            ps[:, :],
                            vT_s[:, dt, lk * P:(lk + 1) * P],
                            WvT_s[:, dt, :],
                            start=(dt == 0), stop=(dt == DT - 1))
                    nc.vector.tensor_copy(
                        V_s[:, lk, :, 0:DV],
                        ps[:, :].rearrange("p (h e) -> p h e", h=H))

                def emit_attn(h, inline_v=False):
                    mt = h // 2
                    avs = [psProj.tile([DV + 1, 512], F32, tag="av", bufs=2,
                                       name=f"av{h}_{j}") for j in range(NB)]
                    for m in range(LKT):
                        if inline_v:
                            emit_vproj(m)
                        ps = psProj.tile([P, LQ], F32, tag="qk", bufs=2)
                        for jb in range(NB):
                            nc.tensor.matmul(
                                ps[:, jb * 512:(jb + 1) * 512],
                                KTz_s[:, h, m * P:(m + 1) * P],
                                QT_s[:, mt, jb * 512:(jb + 1) * 512],
                                start=True, stop=True)
                        pt = pTp.tile([P, LQ], BF16, tag="pT")
                        nc.scalar.activation(
                            out=pt[:, :], in_=ps[:, :], func=AF.Exp,
                            bias=mb_s[:, h * LKT + m:h * LKT + m + 1],
                            scale=1.0 / 8.0)
                        for jb in range(NB):
                            nc.tensor.matmul(
                                avs[jb][:, :],
                                V_s[:, m, h, :],
                                pt[:, jb * 512:(jb + 1) * 512],
                                start=(m == 0), stop=(m == LKT - 1))
                    po = (h % 2) * DV
                    for jb in range(NB):
                        rc = rcp.tile([1, 512], F32, tag="rc")
                        nc.vector.tensor_copy(rc, avs[jb][DV:DV + 1, :])
                        stg = bcsp.tile([DV, 512], F32, tag="stg")
                        nc.vector.tensor_copy(stg, avs[jb][0:DV, :])
                        rcd = dramp.tile([1, 512], F32, tag="rcd",
                                         name=f"rcd{h}_{jb}")
                        nc.sync.dma_start(out=rcd, in_=rc[0:1, :])
                        bcs = bcsp.tile([DV, 512], F32, tag="bcs")
                        nc.gpsimd.dma_start(out=bcs, in_=_bcast(rcd[0:1, :], DV))
                        nc.vector.reciprocal_approx_fast(out=bcs, in_=bcs)
                        nc.vector.tensor_mul(
                            xbf[po:po + DV, mt, jb * 512:(jb + 1) * 512],
                            stg[:, :], bcs[:, :])
                    if h % 2 == 1:
                        # channel tile kt complete: residual + stat inputs
                        kt = h // 2
                        nc.gpsimd.tensor_add(xbf[:, kt, :], xbf[:, kt, :],
                                             qresT_s[:, kt, :])
                        nc.gpsimd.tensor_mul(x2bf[:, kt, :], xbf[:, kt, :],
                                             xbf[:, kt, :])


                emit_kproj(0)
                emit_qproj(0)
                emit_attn(0, inline_v=True)
                emit_attn(1)
                for mt in range(1, DT):
                    emit_kproj(mt)
                    emit_qproj(mt)
                    emit_attn(2 * mt)
                    emit_attn(2 * mt + 1)

            # ============ phase C: LN0 -> fc -> LN1 (all ^T, bf16) ========
            with (
                tc.tile_pool(name="lnp", bufs=1) as lnp,
                tc.tile_pool(name="chain", bufs=2) as chain,
                tc.tile_pool(name="bcB", bufs=4) as bcB,
                tc.tile_pool(name="wfc", bufs=1) as wfc,
            ):
                outT = lnp.tile([P, DT, LQ], F32)
                y2bf = lnp.tile([P, DT, LQ], BF16)

                def ln_half(xb, x2b, g_idx, b_idx, nh, label, final_out=None):
                    """One 512-query half of a transposed LayerNorm, applied
                    in place on the bf16 tile xb."""
                    sl = slice(nh * 512, (nh + 1) * 512)
                    s1 = psProj.tile([1, 512], F32, tag="proj",
                                     name=f"s1{label}{nh}")
                    s2 = psProj.tile([1, 512], F32, tag="proj",
                                     name=f"s2{label}{nh}")
                    for kt in range(DT):
                        nc.tensor.matmul(s1[:, :], ones_st[:, :],
                                         xb[:, kt, sl],
                                         start=(kt == 0), stop=(kt == DT - 1))
                        nc.tensor.matmul(s2[:, :], ones_st[:, :],
                                         x2b[:, kt, sl],
                                         start=(kt == 0), stop=(kt == DT - 1))
                    mu = chain.tile([1, 512], F32, tag="mu")
                    nc.vector.tensor_scalar_mul(mu, s1[:, :], 1.0 / D)
                    var = chain.tile([1, 512], F32, tag="var")
                    nc.vector.tensor_mul(var, mu, mu)
                    msq = chain.tile([1, 512], F32, tag="msq")
                    nc.vector.tensor_scalar_mul(msq, s2[:, :], 1.0 / D)
                    nc.vector.tensor_sub(var, msq, var)
                    nc.scalar.activation(out=var, in_=var, func=AF.Sqrt,
                                         bias=eps_t[0:1, :])
                    rstd = chain.tile([1, 512], F32, tag="rstd")
                    nc.vector.reciprocal_approx_fast(out=rstd, in_=var)
                    mrb = chain.tile([1, 2, 512], BF16, tag="mrb")
                    nc.vector.tensor_copy(mrb[:, 0, :], mu[0:1, :])
                    nc.vector.tensor_copy(mrb[:, 1, :], rstd[0:1, :])
                    mu_b = psProj.tile([P, 512], F32, tag="av",
                                       name=f"mb{label}{nh}")
                    nc.tensor.matmul(mu_b[:, :], ones_r1[:, :], mrb[:, 0, :],
                                     start=True, stop=True)
                    rstd_b = psProj.tile([P, 512], F32, tag="av",
                                         name=f"rb{label}{nh}")
                    nc.tensor.matmul(rstd_b[:, :], ones_r1[:, :], mrb[:, 1, :],
                                     start=True, stop=True)
                    mu_bb = bcB.tile([P, 512], BF16, tag="bc",
                                     name=f"mbb{label}{nh}")
                    nc.vector.tensor_copy(mu_bb, mu_b[:, :])
                    rstd_bb = bcB.tile([P, 512], BF16, tag="bc",
                                       name=f"rbb{label}{nh}")
                    nc.vector.tensor_copy(rstd_bb, rstd_b[:, :])
                    for kt in range(DT):
                        nc.vector.tensor_sub(xb[:, kt, sl], xb[:, kt, sl],
                                             mu_bb[:, :])
                        nc.vector.tensor_mul(xb[:, kt, sl], xb[:, kt, sl],
                                             rstd_bb[:, :])
                        if final_out is not None:
                            nc.vector.tensor_scalar(
                                out=final_out[:, kt, sl], in0=xb[:, kt, sl],
                                scalar1=gbT[:, g_idx, kt:kt + 1],
                                scalar2=gbT[:, b_idx, kt:kt + 1],
                                op0=Alu.mult, op1=Alu.add)
                        else:
                            nc.vector.tensor_scalar(
                                out=xb[:, kt, sl], in0=xb[:, kt, sl],
                                scalar1=gbT[:, g_idx, kt:kt + 1],
                                scalar2=gbT[:, b_idx, kt:kt + 1],
                                op0=Alu.mult, op1=Alu.add)

                fcwT_s = wfc.tile([P, DT, D], BF16)
                nc.sync.dma_start(out=fcwT_s, in_=fcwT[:, :, :])
                ybf = statin.tile([P, DT, LQ], BF16, tag="x2bf")

                # LN0 both halves (in place on xbf = LN0 output, bf16)
                for nh in range(NB):
                    ln_half(xbf, x2bf, 0, 1, nh, "a")
                for nh in range(NB):
                    sl = slice(nh * 512, (nh + 1) * 512)
                    # fc for this half; y = fc + fc_b + LN0 residual (bf16)
                    for m in range(DT):
                        ps = psProj.tile([P, 512], F32, tag="av",
                                         name=f"fc{m}_{nh}")
                        for dt in range(DT):
                            nc.tensor.matmul(
                                ps[:, :],
                                fcwT_s[:, dt, m * P:(m + 1) * P],
                                xbf[:, dt, sl],
                                start=(dt == 0), stop=(dt == DT - 1))
                        nc.vector.tensor_scalar_add(ybf[:, m, sl], ps[:, :],
                                                    gbT[:, 4, m:m + 1])
                        nc.gpsimd.tensor_add(ybf[:, m, sl], ybf[:, m, sl],
                                             xbf[:, m, sl])
                        nc.vector.tensor_mul(y2bf[:, m, sl], ybf[:, m, sl],
                                             ybf[:, m, sl])
                    # LN1 on this half
                    ln_half(ybf, y2bf, 2, 3, nh, "b", final_out=outT)
                    for kt in range(DT):
                        nc.sync.dma_start(out=out[:, kt, sl],
                                          in_=outT[:, kt, sl])


def _build():
    if "nc" in _CACHE:
        return _CACHE["nc"]
    nc = bacc.Bacc(None, target_bir_lowering=False, debug=False)
    with tile.TileContext(nc) as tc:
        _emit(nc, tc)
    nc.compile()
    _CACHE["nc"] = nc
    return nc


def _prep_in_maps(q, k, v, mask, Wq, Wk, Wv, fc_w, fc_b, g0, b0, g1, b1):
    q = np.asarray(q, np.float32)
    k = np.asarray(k, np.float32)
    v = np.asarray(v, np.float32)
    mask = np.asarray(mask)
    bf = mybir.dt.np(BF16)

    def ptile(a):
        # [n, m] -> transpose -> [m(=tiles*128), n] -> [128, tiles, n]
        t = np.asarray(a, np.float32).T
        return np.ascontiguousarray(
            t.reshape(DT, P, t.shape[1]).transpose(1, 0, 2))

    WqTh = ptile(Wq).astype(bf)
    WkTh = ptile(Wk).astype(bf)
    WvTh = ptile(Wv).astype(bf)
    fcwTh = ptile(fc_w).astype(bf)
    vecs = np.stack([np.asarray(x, np.float32).reshape(DT, P).T
                     for x in (g0, b0, g1, b1, fc_b)])
    vecs = np.ascontiguousarray(vecs)

    in_maps = []
    for c in range(NCORES):
        b = c // 2
        r0 = (c % 2) * LQ
        qTb = ptile(q[b][r0:r0 + LQ]).astype(bf)
        kTb = ptile(k[b]).astype(bf)
        vTb = ptile(v[b]).astype(bf)
        qrTb = ptile(q[b][r0:r0 + LQ]).astype(bf)
        mbh = np.zeros((P, H, LKT), np.float32)
        for h in range(H):
            mh = mask[h * B + b].reshape(LKT, P).T  # [p, tile]
            mbh[:, h, :] = np.where(mh == 0, np.float32(NEG), np.float32(0.0))
        in_maps.append({
            "qT": qTb, "kT": kTb, "vT": vTb, "qresT": qrTb,
            "WqT": WqTh, "WkT": WkTh, "WvT": WvTh, "fcwT": fcwTh,
            "mb": np.ascontiguousarray(mbh.reshape(P, H * LKT)),
            "vecs": vecs,
        })
    return in_maps


def kernel(q, k, v, mask, Wq, Wk, Wv, fc_w, fc_b, g0, b0, g1, b1):
    in_maps = _prep_in_maps(q, k, v, mask, Wq, Wk, Wv, fc_w, fc_b,
                            g0, b0, g1, b1)
    nc = _build()
    res = run_bass_kernel_spmd(nc, in_maps, core_ids=list(range(NCORES)))
    outf = np.empty((B, L, D), np.float32)
    for c in range(NCORES):
        b = c // 2
        r0 = (c % 2) * LQ
        o = res.results[c]["out"]  # [128, DT, LQ]
        outf[b, r0:r0 + LQ, :] = o.transpose(2, 1, 0).reshape(LQ, D)
    return outf
